# revision 25
# baseline (speedup 1.0000x reference)
"""AnomalyAttention (two causal attentions per (b,h)) on 8 TRN2 NeuronCores.

Sharding: B*H = 16 (batch, head) pairs -> 2 pairs per core. Each core runs
4 independent causal attentions (time + channel for each of its 2 pairs).
No cross-core communication.

v2 layout ("transposed PV", quarter passes, dual-engine exp, host norm):
  - Queries processed in 4 quarter-passes of 512 per pair (PSUM budget:
    3x2 banks of score tiles in flight + 2 banks of PV accumulators).
  - Per (pass, key-tile) chunk both attention types share one score tile
    [128, 1024]: t0 at cols [0:w], t1 at [512:512+w]. One exp instruction
    covers both halves (merged) when the gap is small enough.
  - exp is split ~50/50 between ACT (exact table exp, bf16 out) and DVE
    (single-op Schraudolph: tensor_scalar fp32 -> int16 computing the bf16
    BIT PATTERN of exp directly; ~3.3% elementwise, cancels in the
    numerator/denominator ratio). Static greedy balance at build time.
  - Diagonal blocks masked post-exp by GPSIMD affine_select (zero fill).
  - PV: out[q, 65] += P^T.T @ V_ext per (j, k); V_ext col 64 is ones so
    col 64 accumulates the softmax denominator.
  - NO device epilogue: raw [128, 4, 65] accumulators are copied
    PSUM->SBUF as bf16 (copy instruction also greedy ACT/DVE) and DMA'd
    out; the host does out[..., :64] / out[..., 64:65].
PSUM discipline: start=True clears has_written for the WHOLE bank, so
exactly one start per bank (first matmul emitted into it).
"""

import math
from contextlib import ExitStack

import ml_dtypes
import numpy as np

import concourse.bacc as bacc
import concourse.mybir as mybir
import concourse.tile as tile
from concourse.bass_utils import run_bass_kernel_spmd

B, L, H, E, D = 2, 2048, 8, 64, 64
NCORES = 8
PAIRS = (B * H) // NCORES          # (b,h) pairs per core = 2
NATT = 2 * PAIRS                   # attentions per core = 4
SCALE = 1.0 / math.sqrt(E)
P = 128                            # partitions / key-tile size
NKT = L // P                       # 16 key tiles
QP = 512                           # quarter-pass query width
NPASS = L // QP                    # 4 passes per pair
DP1 = D + 1                        # value cols + denominator ones-column
F32 = mybir.dt.float32
I16 = mybir.dt.int16
BF16 = mybir.dt.bfloat16

LOG2E = 1.4426950408889634
# int16 Schraudolph: bf16 bits of exp(SCALE*s) = round(s*SCH_A + SCH_B)
SCH_C = 5.6
SCH_A = SCALE * LOG2E * (1 << 7)
SCH_B = float((127 << 7) - SCH_C)

HALF = L // 2
# fp8 DoubleRow scores for passes 2-3 (queries >= 1024): those rows average
# over >=1024 softmax terms, so the ~e4m3 quantization noise on Q/K washes
# out (measured ~6e-3 max-normalized); early rows keep bf16 exactness.
USE_FP8 = True
FP8 = mybir.dt.float8e4

_CACHE = {}

# engine cost models (ns) for the static greedy exp balance; for aligned
# (gapped) chunks the exp either covers the gap (merged) or splits in two
def _act_cost(w, toff):
    if toff == w:
        return (2 * w + 352) / 1.2, True
    merged = (toff + w + 352) / 1.2
    split = (2 * w + 704) / 1.2
    return (merged, True) if merged <= split else (split, False)


def _dve_cost(w, toff):
    if toff == w:
        return 2 * w * 1.04 + 150, True
    merged = (toff + w) * 1.04 + 150
    split = 2 * w * 1.04 + 300
    return (merged, True) if merged <= split else (split, False)


def _build_nc():
    nc = bacc.Bacc()
    qt = nc.declare_dram_parameter("qt", [P, PAIRS, L], BF16, isOutput=False)
    # kt zero-padded to full 128 contraction rows per type: rows 0-63 hold
    # kt_time (t=0) / zeros (t=1), rows 64-127 zeros / kt_chan. This lets the
    # score matmul's MOVING operand (qt) span all 128 partitions -> full SBUF
    # port bandwidth. The extra zero contraction rows are free: matmul cost
    # depends only on output columns.
    kt = nc.declare_dram_parameter("kt", [P, 2, PAIRS, L], BF16, isOutput=False)
    if USE_FP8:
        # E-dim folded as [32 partitions, 2 sub-rows] for DoubleRow
        qt8 = nc.declare_dram_parameter("qt8", [64, PAIRS, 2, HALF], FP8, isOutput=False)
        kt8 = nc.declare_dram_parameter("kt8", [64, PAIRS, 2, L], FP8, isOutput=False)
    else:
        qt8 = kt8 = None
    ve = nc.declare_dram_parameter("ve", [P, NATT, NKT, DP1], BF16, isOutput=False)
    # raw accumulators, partition-major so output DMAs are dim-aligned:
    # out[qq, g, pss, t, jj*DP1+d] = PV accum for attention 2g+t, query
    # 128*(4*pss+jj)+qq, value-col d (d=64 is the softmax denominator)
    out = nc.declare_dram_parameter(
        "out", [P, PAIRS, NPASS, 2, 4 * DP1], BF16, isOutput=True
    )

    with tile.TileContext(nc) as tc:
        with ExitStack() as ctx:
            _body(ctx, tc, qt, kt, qt8, kt8, ve, out)
    nc.finalize()
    return nc


def _body(ctx, tc, qt, kt, qt8, kt8, ve, out):
    nc = tc.nc
    Exp = mybir.ActivationFunctionType.Exp
    Copy = mybir.ActivationFunctionType.Copy

    persist = ctx.enter_context(tc.tile_pool(name="persist", bufs=1))
    s_pool = ctx.enter_context(tc.tile_pool(name="s_pool", bufs=3, space="PSUM"))
    pv_pool = ctx.enter_context(tc.tile_pool(name="pv_pool", bufs=1, space="PSUM"))
    p_pool = ctx.enter_context(tc.tile_pool(name="p_pool", bufs=3))
    ob_pool = ctx.enter_context(tc.tile_pool(name="ob_pool", bufs=3))

    qt_sb = persist.tile([P, PAIRS, L], BF16)
    kt_sb = persist.tile([P, 2, PAIRS, L], BF16)
    ve_sb = persist.tile([P, NATT, NKT, DP1], BF16)
    if USE_FP8:
        qt8_sb = persist.tile([64, PAIRS, 2, HALF], FP8)
        kt8_sb = persist.tile([64, PAIRS, 2, L], FP8)

    # staged input DMA in need-order across three trigger queues, so each
    # pass's operands land just before the PE reaches them
    nc.gpsimd.dma_start(out=kt_sb[:, :, 0, 0:P], in_=kt[:, :, 0, 0:P])
    nc.scalar.dma_start(out=qt_sb[:, 0, 0:QP], in_=qt[:, 0, 0:QP])
    nc.sync.dma_start(out=kt_sb[:, :, 0, P:QP], in_=kt[:, :, 0, P:QP])
    nc.gpsimd.dma_start(out=ve_sb[:, 0], in_=ve[:, 0])
    nc.gpsimd.dma_start(out=ve_sb[:, 1], in_=ve[:, 1])
    nc.scalar.dma_start(out=qt_sb[:, 0, QP:L], in_=qt[:, 0, QP:L])
    nc.sync.dma_start(out=kt_sb[:, :, 0, QP:2 * QP], in_=kt[:, :, 0, QP:2 * QP])
    nc.sync.dma_start(out=kt_sb[:, :, 0, 2 * QP:L], in_=kt[:, :, 0, 2 * QP:L])
    if USE_FP8:
        nc.scalar.dma_start(out=qt8_sb[:, 0], in_=qt8[:, 0])
        nc.sync.dma_start(out=kt8_sb[:, 0], in_=kt8[:, 0])
    nc.sync.dma_start(out=kt_sb[:, :, 1], in_=kt[:, :, 1])
    nc.scalar.dma_start(out=qt_sb[:, 1], in_=qt[:, 1])
    nc.gpsimd.dma_start(out=ve_sb[:, 2], in_=ve[:, 2])
    nc.gpsimd.dma_start(out=ve_sb[:, 3], in_=ve[:, 3])
    if USE_FP8:
        nc.scalar.dma_start(out=qt8_sb[:, 1], in_=qt8[:, 1])
        nc.sync.dma_start(out=kt8_sb[:, 1], in_=kt8[:, 1])

    # warm the ACT exp table (emitted after the DMA triggers so those fire
    # first; the table load then hides under the input transfers)
    warm = persist.tile([1, 8], F32)
    nc.vector.memset(warm, 0.0)
    nc.scalar.activation(warm, warm, Exp)

    # warm the PE HAM clock gate during the input-DMA wait: ~6us of dummy
    # matmuls flip the PE to 2.4GHz before real work arrives (the activity
    # monitor needs ~3.4us of sustained busy; idle gaps < 3.4us keep it warm)
    dm = persist.tile([P, 256], BF16)
    nc.vector.memset(dm, 0.0)
    wps = s_pool.tile([P, 2 * QP], F32, tag="s", name="warm_s")
    for _ in range(40):
        nc.tensor.matmul(
            wps[:, 0:256], lhsT=dm[:, 0:P], rhs=dm,
            start=True, stop=True, skip_group_check=True,
        )

    # static greedy balance state: accumulated busy ns per exp engine
    acc = {"act": 0.0, "dve": 0.0}

    def pick_engine(cost_act, cost_dve):
        # assign to the engine minimizing the resulting makespan
        if max(acc["act"] + cost_act, acc["dve"]) <= max(
            acc["dve"] + cost_dve, acc["act"]
        ):
            acc["act"] += cost_act
            return "act"
        acc["dve"] += cost_dve
        return "dve"

    def emit_exp(s_t, w, toff, diag, force_act=False):
        """exp both halves of a combined score tile [t0: 0..w, t1:
        toff..toff+w]; merged into one instruction when the gap is small.
        Returns pT (bf16 view)."""
        ca, ma = _act_cost(w, toff)
        cv, mv = _dve_cost(w, toff)
        if force_act:
            acc["act"] += ca
            eng = "act"
        else:
            eng = pick_engine(ca, cv)
        merged = ma if eng == "act" else mv
        ranges = [(0, toff + w)] if merged else [(0, w), (toff, toff + w)]
        if eng == "act":
            pT = p_pool.tile([P, 2 * QP], BF16, tag="pa", name="pa")
            for lo, hi in ranges:
                nc.scalar.activation(pT[:, lo:hi], s_t[:, lo:hi], Exp, scale=SCALE)
            pTb = pT
        else:
            pTi = p_pool.tile([P, 2 * QP], I16, tag="pi", name="pi")
            for lo, hi in ranges:
                nc.vector.tensor_scalar(
                    out=pTi[:, lo:hi], in0=s_t[:, lo:hi],
                    scalar1=float(SCH_A), scalar2=float(SCH_B),
                    op0=mybir.AluOpType.mult, op1=mybir.AluOpType.add,
                )
            pTb = pTi.bitcast(BF16)
        if diag:
            # zero the strictly-upper triangle of the diagonal 128-block
            # (query < key) for both types
            for lo in (0, toff):
                nc.gpsimd.affine_select(
                    out=pTb[:, lo:lo + P], in_=pTb[:, lo:lo + P],
                    compare_op=mybir.AluOpType.is_ge, fill=0.0,
                    base=0, channel_multiplier=-1, pattern=[[1, P]],
                )
        return pTb

    for g in range(PAIRS):
        for pss in range(NPASS):
            q0 = pss * QP
            j0 = 4 * pss
            kmax = 4 * (pss + 1)
            # one PV accumulator tile for both types: t0 in bank 0 (cols
            # 0:260), t1 in bank 1 (cols 512:772)
            pv = pv_pool.tile([P, 2, QP], F32, tag="pv", name="pv")
            started = [False, False]

            def emit_pv(t, k, qlo, w, toff, pTb):
                a = 2 * g + t
                for j in range(max(j0, k), j0 + 4):
                    col = P * j - qlo + t * toff
                    first = not started[t]
                    started[t] = True
                    nc.tensor.matmul(
                        pv[:, t, (j - j0) * DP1:(j - j0 + 1) * DP1],
                        lhsT=pTb[:, col:col + P],
                        rhs=ve_sb[:, a, k, :],
                        start=first,
                        stop=(k == j),
                        skip_group_check=True,
                    )

            pend = []
            for k in range(kmax):
                qlo = max(q0, P * k)
                w = q0 + QP - qlo
                diag = qlo == P * k
                s_t = s_pool.tile([P, 2 * QP], F32, tag="s", name="s")
                fp8 = USE_FP8 and pss >= 2
                # bf16 chunks pack gap-free: t0 at [0:w], t1 at [w:2w], with
                # matmul outputs split at absolute 512-col PSUM bank
                # boundaries (first matmul into each bank carries start=True,
                # which clears the whole bank's has_written; followers use
                # False). fp8 DoubleRow matmuls fault on such start=False
                # continuation segments, so fp8 chunks use 512-aligned
                # placement (t1 at [512:512+w], a gap for partial widths).
                toff = QP if fp8 else w
                if fp8:
                    segs = [(0, 0, w, True), (1, QP, QP + w, True)]
                else:
                    segs = [(0, 0, w, True)]
                    lo = w
                    while lo < 2 * w:
                        hi = min(2 * w, (lo // QP + 1) * QP)
                        segs.append((1, lo, hi, lo % QP == 0))
                        lo = hi
                for t, lo, hi, bank_first in segs:
                    c0 = qlo + lo - t * toff
                    c1 = qlo + hi - t * toff
                    if fp8:
                        nc.tensor.matmul(
                            s_t[:, lo:hi],
                            lhsT=kt8_sb[32 * t:32 * (t + 1), g, :, P * k:P * (k + 1)],
                            rhs=qt8_sb[32 * t:32 * (t + 1), g, :, c0 - HALF:c1 - HALF],
                            start=bank_first,
                            stop=True,
                            perf_mode=mybir.MatmulPerfMode.DoubleRow,
                            skip_group_check=True,
                        )
                    else:
                        nc.tensor.matmul(
                            s_t[:, lo:hi],
                            lhsT=kt_sb[:, t, g, P * k:P * (k + 1)],
                            rhs=qt_sb[:, g, c0:c1],
                            start=bank_first,
                            stop=True,
                            skip_group_check=True,
                        )
                # PV trails two chunks behind its exp so PE instructions
                # enter the queue with satisfied deps
                if len(pend) == 2:
                    kk, qq, ww, to, pp = pend.pop(0)
                    emit_pv(0, kk, qq, ww, to, pp)
                    emit_pv(1, kk, qq, ww, to, pp)
                # queries 0:511 average over few softmax terms, so the
                # Schraudolph ~3% element error would not cancel there:
                # keep the first key-tile of pass 0 on the exact ACT exp
                pTb = emit_exp(s_t, w, toff, diag, force_act=(pss == 0 and k == 0))
                pend.append((k, qlo, w, toff, pTb))
            for kk, qq, ww, to, pp in pend:
                emit_pv(0, kk, qq, ww, to, pp)
                emit_pv(1, kk, qq, ww, to, pp)

            # raw accumulators (incl denominator col 64) -> SBUF bf16 -> DRAM
            last = g == PAIRS - 1 and pss == NPASS - 1
            ob = ob_pool.tile([P, 2, 4 * DP1], BF16, tag="ob", name="ob")
            pv_v = pv[:, :, 0:4 * DP1]
            cost_a = (2 * 4 * DP1 + 352) / 1.2
            cost_v = 2 * 4 * DP1 * 1.04 + 150
            if pick_engine(cost_a, cost_v) == "act":
                nc.scalar.activation(ob, pv_v, Copy)
            else:
                nc.vector.tensor_copy(out=ob, in_=pv_v)
            eng = nc.gpsimd if last else (nc.sync if pss % 2 == 0 else nc.scalar)
            eng.dma_start(out=out[:, g, pss], in_=ob)


def _host_shard(inputs):
    """Build the 8 per-core input maps from full inputs (host-side numpy)."""
    q_t = np.asarray(inputs["queries_time"], dtype=np.float32)
    k_t = np.asarray(inputs["keys_time"], dtype=np.float32)
    v_t = np.asarray(inputs["values_time"], dtype=np.float32)
    q_c = np.asarray(inputs["queries_channel"], dtype=np.float32)
    k_c = np.asarray(inputs["keys_channel"], dtype=np.float32)
    v_c = np.asarray(inputs["values_channel"], dtype=np.float32)

    bf16 = ml_dtypes.bfloat16
    fp8 = ml_dtypes.float8_e4m3
    in_maps = []
    for c in range(NCORES):
        vem = np.empty((P, NATT, NKT, DP1), np.float32)
        qtm = np.empty((P, PAIRS, L), np.float32)
        ktm = np.zeros((P, 2, PAIRS, L), np.float32)
        qt8m = np.empty((64, PAIRS, 2, HALF), np.float32)
        kt8m = np.empty((64, PAIRS, 2, L), np.float32)
        for g in range(PAIRS):
            p = PAIRS * c + g
            b, h = divmod(p, H)
            qtm[:64, g, :] = q_t[b, :, h, :].T
            qtm[64:, g, :] = q_c[b, :, h, :].T
            ktm[:64, 0, g, :] = k_t[b, :, h, :].T
            ktm[64:, 1, g, :] = k_c[b, :, h, :].T
            if USE_FP8:
                for t, (qf, kf) in enumerate(((q_t, k_t), (q_c, k_c))):
                    # E-index e -> partition 32*t + e%32, sub-row e//32
                    qT = qf[b, HALF:, h, :].T.reshape(2, 32, HALF)
                    kT = kf[b, :, h, :].T.reshape(2, 32, L)
                    qt8m[32 * t:32 * (t + 1), g] = qT.transpose(1, 0, 2)
                    kt8m[32 * t:32 * (t + 1), g] = kT.transpose(1, 0, 2)
            for t, v_full in enumerate((v_t, v_c)):
                a = 2 * g + t
                vem[:, a, :, :D] = (
                    v_full[b, :, h, :].reshape(NKT, P, D).transpose(1, 0, 2)
                )
                vem[:, a, :, D] = 1.0
        m = {
            "qt": np.ascontiguousarray(qtm).astype(bf16),
            "kt": np.ascontiguousarray(ktm).astype(bf16),
            "ve": np.ascontiguousarray(vem).astype(bf16),
        }
        if USE_FP8:
            m["qt8"] = np.ascontiguousarray(qt8m).astype(fp8)
            m["kt8"] = np.ascontiguousarray(kt8m).astype(fp8)
        in_maps.append(m)
    return in_maps


def _run(in_maps, trace=False):
    if "nc" not in _CACHE:
        _CACHE["nc"] = _build_nc()
    return run_bass_kernel_spmd(
        _CACHE["nc"], in_maps, core_ids=list(range(NCORES)), trace=trace
    )


def kernel(**inputs):
    in_maps = _host_shard(inputs)
    res = _run(in_maps, trace=False)
    v_time = np.empty((B, L, H, D), np.float32)
    v_chan = np.empty((B, L, H, D), np.float32)
    for c in range(NCORES):
        o = np.asarray(res.results[c]["out"]).astype(np.float32)
        # [P, PAIRS, NPASS, 2, 4*DP1] -> [P, PAIRS, 2, NKT, DP1]
        o = o.reshape(P, PAIRS, NPASS, 2, 4, DP1)
        o = o.transpose(1, 3, 0, 2, 4, 5).reshape(PAIRS, 2, P, NKT, DP1)
        o = o[..., :D] / o[..., D:DP1]  # host-side softmax normalization
        for g in range(PAIRS):
            p = PAIRS * c + g
            b, h = divmod(p, H)
            # q = 128*j + qq lives at o[g, t, qq, j, :]
            v_time[b, :, h, :] = o[g, 0].transpose(1, 0, 2).reshape(L, D)
            v_chan[b, :, h, :] = o[g, 1].transpose(1, 0, 2).reshape(L, D)
    return v_time, v_chan


# revision 29
# speedup vs baseline: 1.2151x; 1.2151x over previous
"""AnomalyAttention (two causal attentions per (b,h)) on 8 TRN2 NeuronCores.

Sharding: B*H = 16 (batch, head) pairs -> 2 pairs per core. Each core runs
4 independent causal attentions (time + channel for each of its 2 pairs).
No cross-core communication.

v2 layout ("transposed PV", quarter passes, dual-engine exp, host norm):
  - Queries processed in 4 quarter-passes of 512 per pair (PSUM budget:
    3x2 banks of score tiles in flight + 2 banks of PV accumulators).
  - Per (pass, key-tile) chunk both attention types share one score tile
    [128, 1024]: t0 at cols [0:w], t1 at [512:512+w]. One exp instruction
    covers both halves (merged) when the gap is small enough.
  - exp is split ~50/50 between ACT (exact table exp, bf16 out) and DVE
    (single-op Schraudolph: tensor_scalar fp32 -> int16 computing the bf16
    BIT PATTERN of exp directly; ~3.3% elementwise, cancels in the
    numerator/denominator ratio). Static greedy balance at build time.
  - Diagonal blocks masked post-exp by GPSIMD affine_select (zero fill).
  - PV: out[q, 65] += P^T.T @ V_ext per (j, k); V_ext col 64 is ones so
    col 64 accumulates the softmax denominator.
  - NO device epilogue: raw [128, 4, 65] accumulators are copied
    PSUM->SBUF as bf16 (copy instruction also greedy ACT/DVE) and DMA'd
    out; the host does out[..., :64] / out[..., 64:65].
PSUM discipline: start=True clears has_written for the WHOLE bank, so
exactly one start per bank (first matmul emitted into it).
"""

import math
from contextlib import ExitStack

import ml_dtypes
import numpy as np

import concourse.bacc as bacc
import concourse.mybir as mybir
import concourse.tile as tile
from concourse.bass_utils import run_bass_kernel_spmd

B, L, H, E, D = 2, 2048, 8, 64, 64
NCORES = 8
PAIRS = (B * H) // NCORES          # (b,h) pairs per core = 2
NATT = 2 * PAIRS                   # attentions per core = 4
SCALE = 1.0 / math.sqrt(E)
P = 128                            # partitions / key-tile size
NKT = L // P                       # 16 key tiles
QP = 512                           # quarter-pass query width
NPASS = L // QP                    # 4 passes per pair
DP1 = D + 1                        # value cols + denominator ones-column
F32 = mybir.dt.float32
I16 = mybir.dt.int16
BF16 = mybir.dt.bfloat16

LOG2E = 1.4426950408889634
# int16 Schraudolph: bf16 bits of exp(SCALE*s) = round(s*SCH_A + SCH_B)
SCH_C = 5.6
SCH_A = SCALE * LOG2E * (1 << 7)
SCH_B = float((127 << 7) - SCH_C)

HALF = L // 2
# fp8 DoubleRow scores for passes 2-3 (queries >= 1024): those rows average
# over >=1024 softmax terms, so the ~e4m3 quantization noise on Q/K washes
# out (measured ~6e-3 max-normalized); early rows keep bf16 exactness.
USE_FP8 = False
FP8 = mybir.dt.float8e4

_CACHE = {}

# engine cost models (ns) for the static greedy exp balance; for aligned
# (gapped) chunks the exp either covers the gap (merged) or splits in two
def _act_cost(w, toff):
    if toff == w:
        return (2 * w + 352) / 1.2, True
    merged = (toff + w + 352) / 1.2
    split = (2 * w + 704) / 1.2
    return (merged, True) if merged <= split else (split, False)


def _dve_cost(w, toff):
    if toff == w:
        return 2 * w * 1.04 + 150, True
    merged = (toff + w) * 1.04 + 150
    split = 2 * w * 1.04 + 300
    return (merged, True) if merged <= split else (split, False)


def _build_nc():
    nc = bacc.Bacc()
    qt = nc.declare_dram_parameter("qt", [P, PAIRS, L], BF16, isOutput=False)
    # kt zero-padded to full 128 contraction rows per type: rows 0-63 hold
    # kt_time (t=0) / zeros (t=1), rows 64-127 zeros / kt_chan. This lets the
    # score matmul's MOVING operand (qt) span all 128 partitions -> full SBUF
    # port bandwidth. The extra zero contraction rows are free: matmul cost
    # depends only on output columns.
    kt = nc.declare_dram_parameter("kt", [P, 2, PAIRS, L], BF16, isOutput=False)
    if USE_FP8:
        # E-dim folded as [32 partitions, 2 sub-rows] for DoubleRow
        qt8 = nc.declare_dram_parameter("qt8", [64, PAIRS, 2, HALF], FP8, isOutput=False)
        kt8 = nc.declare_dram_parameter("kt8", [64, PAIRS, 2, L], FP8, isOutput=False)
    else:
        qt8 = kt8 = None
    ve = nc.declare_dram_parameter("ve", [P, NATT, NKT, DP1], BF16, isOutput=False)
    # raw accumulators, partition-major so output DMAs are dim-aligned:
    # out[qq, g, pss, t, jj*DP1+d] = PV accum for attention 2g+t, query
    # 128*(4*pss+jj)+qq, value-col d (d=64 is the softmax denominator)
    out = nc.declare_dram_parameter(
        "out", [P, PAIRS, NPASS, 2, 4 * DP1], BF16, isOutput=True
    )

    with tile.TileContext(nc) as tc:
        with ExitStack() as ctx:
            _body(ctx, tc, qt, kt, qt8, kt8, ve, out)
    nc.finalize()
    return nc


def _body(ctx, tc, qt, kt, qt8, kt8, ve, out):
    nc = tc.nc
    Exp = mybir.ActivationFunctionType.Exp
    Copy = mybir.ActivationFunctionType.Copy

    persist = ctx.enter_context(tc.tile_pool(name="persist", bufs=1))
    s_pool = ctx.enter_context(tc.tile_pool(name="s_pool", bufs=3, space="PSUM"))
    pv_pool = ctx.enter_context(tc.tile_pool(name="pv_pool", bufs=1, space="PSUM"))
    p_pool = ctx.enter_context(tc.tile_pool(name="p_pool", bufs=3))
    ob_pool = ctx.enter_context(tc.tile_pool(name="ob_pool", bufs=3))

    qt_sb = persist.tile([P, PAIRS, L], BF16)
    kt_sb = persist.tile([P, 2, PAIRS, L], BF16)
    ve_sb = persist.tile([P, NATT, NKT, DP1], BF16)
    if USE_FP8:
        qt8_sb = persist.tile([64, PAIRS, 2, HALF], FP8)
        kt8_sb = persist.tile([64, PAIRS, 2, L], FP8)

    # staged input DMA in need-order across three trigger queues, so each
    # pass's operands land just before the PE reaches them
    nc.gpsimd.dma_start(out=kt_sb[:, :, 0, 0:P], in_=kt[:, :, 0, 0:P])
    nc.scalar.dma_start(out=qt_sb[:, 0, 0:QP], in_=qt[:, 0, 0:QP])
    nc.sync.dma_start(out=kt_sb[:, :, 0, P:QP], in_=kt[:, :, 0, P:QP])
    nc.gpsimd.dma_start(out=ve_sb[:, 0], in_=ve[:, 0])
    nc.gpsimd.dma_start(out=ve_sb[:, 1], in_=ve[:, 1])
    nc.scalar.dma_start(out=qt_sb[:, 0, QP:L], in_=qt[:, 0, QP:L])
    nc.sync.dma_start(out=kt_sb[:, :, 0, QP:2 * QP], in_=kt[:, :, 0, QP:2 * QP])
    nc.sync.dma_start(out=kt_sb[:, :, 0, 2 * QP:L], in_=kt[:, :, 0, 2 * QP:L])
    if USE_FP8:
        nc.scalar.dma_start(out=qt8_sb[:, 0], in_=qt8[:, 0])
        nc.sync.dma_start(out=kt8_sb[:, 0], in_=kt8[:, 0])
    nc.sync.dma_start(out=kt_sb[:, :, 1], in_=kt[:, :, 1])
    nc.scalar.dma_start(out=qt_sb[:, 1], in_=qt[:, 1])
    nc.gpsimd.dma_start(out=ve_sb[:, 2], in_=ve[:, 2])
    nc.gpsimd.dma_start(out=ve_sb[:, 3], in_=ve[:, 3])
    if USE_FP8:
        nc.scalar.dma_start(out=qt8_sb[:, 1], in_=qt8[:, 1])
        nc.sync.dma_start(out=kt8_sb[:, 1], in_=kt8[:, 1])

    # warm the ACT exp table (emitted after the DMA triggers so those fire
    # first; the table load then hides under the input transfers)
    warm = persist.tile([1, 8], F32)
    nc.vector.memset(warm, 0.0)
    nc.scalar.activation(warm, warm, Exp)

    # warm the PE HAM clock gate during the input-DMA wait: ~6us of dummy
    # matmuls flip the PE to 2.4GHz before real work arrives (the activity
    # monitor needs ~3.4us of sustained busy; idle gaps < 3.4us keep it warm)
    dm = persist.tile([P, 256], BF16)
    nc.vector.memset(dm, 0.0)
    wps = s_pool.tile([P, 2 * QP], F32, tag="s", name="warm_s")
    for _ in range(24):
        nc.tensor.matmul(
            wps[:, 0:256], lhsT=dm[:, 0:P], rhs=dm,
            start=True, stop=True, skip_group_check=True,
        )

    # static greedy balance state: accumulated busy ns per exp engine
    acc = {"act": 0.0, "dve": 0.0}

    def pick_engine(cost_act, cost_dve):
        # assign to the engine minimizing the resulting makespan
        if max(acc["act"] + cost_act, acc["dve"]) <= max(
            acc["dve"] + cost_dve, acc["act"]
        ):
            acc["act"] += cost_act
            return "act"
        acc["dve"] += cost_dve
        return "dve"

    def emit_exp(s_t, w, toff, diag, force_act=False):
        """exp both halves of a combined score tile [t0: 0..w, t1:
        toff..toff+w]; merged into one instruction when the gap is small.
        Returns pT (bf16 view)."""
        ca, ma = _act_cost(w, toff)
        cv, mv = _dve_cost(w, toff)
        if force_act:
            acc["act"] += ca
            eng = "act"
        else:
            eng = pick_engine(ca, cv)
        merged = ma if eng == "act" else mv
        ranges = [(0, toff + w)] if merged else [(0, w), (toff, toff + w)]
        if eng == "act":
            pT = p_pool.tile([P, 2 * QP], BF16, tag="pa", name="pa")
            for lo, hi in ranges:
                nc.scalar.activation(pT[:, lo:hi], s_t[:, lo:hi], Exp, scale=SCALE)
            pTb = pT
        else:
            pTi = p_pool.tile([P, 2 * QP], I16, tag="pi", name="pi")
            for lo, hi in ranges:
                nc.vector.tensor_scalar(
                    out=pTi[:, lo:hi], in0=s_t[:, lo:hi],
                    scalar1=float(SCH_A), scalar2=float(SCH_B),
                    op0=mybir.AluOpType.mult, op1=mybir.AluOpType.add,
                )
            pTb = pTi.bitcast(BF16)
        if diag:
            # zero the strictly-upper triangle of the diagonal 128-block
            # (query < key) for both types
            for lo in (0, toff):
                nc.gpsimd.affine_select(
                    out=pTb[:, lo:lo + P], in_=pTb[:, lo:lo + P],
                    compare_op=mybir.AluOpType.is_ge, fill=0.0,
                    base=0, channel_multiplier=-1, pattern=[[1, P]],
                )
        return pTb

    def emit_copy(pv, g, pss, jlo, jhi, last):
        """pv accumulators for j-tiles [jlo, jhi) of this pass -> SBUF bf16
        -> DRAM (incl denominator col 64; the host divides)."""
        n = (jhi - jlo) * DP1
        ob = ob_pool.tile([P, 2, n], BF16, tag=f"ob{n}", name="ob")
        pv_v = pv[:, :, jlo * DP1:jhi * DP1]
        cost_a = (2 * n + 352) / 1.2
        cost_v = 2 * n * 1.04 + 150
        if pick_engine(cost_a, cost_v) == "act":
            nc.scalar.activation(ob, pv_v, Copy)
        else:
            nc.vector.tensor_copy(out=ob, in_=pv_v)
        eng = nc.gpsimd if last else (nc.sync if pss % 2 == 0 else nc.scalar)
        eng.dma_start(
            out=out[:, g, pss, :, jlo * DP1:jhi * DP1], in_=ob
        )

    # one flat, globally-pipelined schedule over all (pair, pass, key-tile)
    # chunks: PV matmuls trail two chunks behind their exp (so PE
    # instructions enter the queue with satisfied deps) and are interleaved
    # between the next chunk's score matmuls so each PV LDWEIGHTS can
    # prefetch under a long score stream.
    class Pass:
        def __init__(self, g, pss):
            self.g, self.pss = g, pss
            self.q0, self.j0 = pss * QP, 4 * pss
            self.kmax = 4 * (pss + 1)
            # one PV accumulator tile for both types: t0 in bank 0 (cols
            # 0:260), t1 in bank 1 (cols 512:772)
            self.pv = pv_pool.tile([P, 2, QP], F32, tag="pv", name="pv")
            self.started = [False, False]

        def emit_pv(self, t, k, qlo, w, toff, pTb):
            a = 2 * self.g + t
            for j in range(max(self.j0, k), self.j0 + 4):
                col = P * j - qlo + t * toff
                first = not self.started[t]
                self.started[t] = True
                nc.tensor.matmul(
                    self.pv[:, t, (j - self.j0) * DP1:(j - self.j0 + 1) * DP1],
                    lhsT=pTb[:, col:col + P],
                    rhs=ve_sb[:, a, k, :],
                    start=first,
                    stop=(k == j),
                    skip_group_check=True,
                )

    chunks = []
    for g in range(PAIRS):
        for pss in range(NPASS):
            for k in range(4 * (pss + 1)):
                chunks.append((g, pss, k))

    pend = []
    cur_pass = None

    def pop_pv(half):
        """Emit one type's trailing PV matmuls (half 0 -> t0, 1 -> t1 and
        retire); called between score segments of the current chunk."""
        if not pend:
            return
        pa, kk, qq, ww, to, pp, done = pend[0]
        pa.emit_pv(half, kk, qq, ww, to, pp)
        if half == 1:
            pend.pop(0)
            last = pa.g == PAIRS - 1 and pa.pss == NPASS - 1
            if last and kk == pa.kmax - 2:
                # final pass: j-tiles 0..2 are already complete after the
                # second-to-last key-tile; flush them early so only a tiny
                # copy+DMA trails the very last matmul
                emit_copy(pa.pv, pa.g, pa.pss, 0, 3, False)
            if done:
                # final key-tile of its pass: accumulators complete, flush
                if last:
                    emit_copy(pa.pv, pa.g, pa.pss, 3, 4, True)
                else:
                    emit_copy(pa.pv, pa.g, pa.pss, 0, 4, False)

    for g, pss, k in chunks:
        if cur_pass is None or (cur_pass.g, cur_pass.pss) != (g, pss):
            cur_pass = Pass(g, pss)
        q0 = pss * QP
        qlo = max(q0, P * k)
        w = q0 + QP - qlo
        diag = qlo == P * k
        s_t = s_pool.tile([P, 2 * QP], F32, tag="s", name="s")
        fp8 = USE_FP8 and pss >= 2
        # bf16 chunks pack gap-free: t0 at [0:w], t1 at [w:2w], with matmul
        # outputs split at absolute 512-col PSUM bank boundaries (first
        # matmul into each bank carries start=True, which clears the whole
        # bank's has_written; followers use False). fp8 DoubleRow matmuls
        # fault on such start=False continuation segments, so fp8 chunks use
        # 512-aligned placement (t1 at [512:512+w], a gap for partial widths).
        toff = QP if fp8 else w
        if fp8:
            segs = [(0, 0, w, True), (1, QP, QP + w, True)]
        else:
            segs = [(0, 0, w, True)]
            lo = w
            while lo < 2 * w:
                hi = min(2 * w, (lo // QP + 1) * QP)
                segs.append((1, lo, hi, lo % QP == 0))
                lo = hi
        emitted_pv = 0
        for t, lo, hi, bank_first in segs:
            c0 = qlo + lo - t * toff
            c1 = qlo + hi - t * toff
            if fp8:
                nc.tensor.matmul(
                    s_t[:, lo:hi],
                    lhsT=kt8_sb[32 * t:32 * (t + 1), g, :, P * k:P * (k + 1)],
                    rhs=qt8_sb[32 * t:32 * (t + 1), g, :, c0 - HALF:c1 - HALF],
                    start=bank_first,
                    stop=True,
                    perf_mode=mybir.MatmulPerfMode.DoubleRow,
                    skip_group_check=True,
                )
            else:
                nc.tensor.matmul(
                    s_t[:, lo:hi],
                    lhsT=kt_sb[:, t, g, P * k:P * (k + 1)],
                    rhs=qt_sb[:, g, c0:c1],
                    start=bank_first,
                    stop=True,
                    skip_group_check=True,
                )
            # interleave the trailing PV work between score segments
            if len(pend) == 2 and emitted_pv < 2:
                pop_pv(emitted_pv)
                emitted_pv += 1
        while len(pend) == 2 and emitted_pv < 2:
            pop_pv(emitted_pv)
            emitted_pv += 1
        # queries 0:511 average over few softmax terms, so the Schraudolph
        # ~3% element error would not cancel there: keep the first key-tile
        # of pass 0 on the exact ACT exp
        pTb = emit_exp(s_t, w, toff, diag, force_act=(pss == 0 and k == 0))
        pend.append(
            (cur_pass, k, qlo, w, toff, pTb, k == cur_pass.kmax - 1)
        )
    while pend:
        pop_pv(0)
        pop_pv(1)


def _host_shard(inputs):
    """Build the 8 per-core input maps from full inputs (host-side numpy)."""
    q_t = np.asarray(inputs["queries_time"], dtype=np.float32)
    k_t = np.asarray(inputs["keys_time"], dtype=np.float32)
    v_t = np.asarray(inputs["values_time"], dtype=np.float32)
    q_c = np.asarray(inputs["queries_channel"], dtype=np.float32)
    k_c = np.asarray(inputs["keys_channel"], dtype=np.float32)
    v_c = np.asarray(inputs["values_channel"], dtype=np.float32)

    bf16 = ml_dtypes.bfloat16
    fp8 = ml_dtypes.float8_e4m3
    in_maps = []
    for c in range(NCORES):
        vem = np.empty((P, NATT, NKT, DP1), np.float32)
        qtm = np.empty((P, PAIRS, L), np.float32)
        ktm = np.zeros((P, 2, PAIRS, L), np.float32)
        qt8m = np.empty((64, PAIRS, 2, HALF), np.float32)
        kt8m = np.empty((64, PAIRS, 2, L), np.float32)
        for g in range(PAIRS):
            p = PAIRS * c + g
            b, h = divmod(p, H)
            qtm[:64, g, :] = q_t[b, :, h, :].T
            qtm[64:, g, :] = q_c[b, :, h, :].T
            ktm[:64, 0, g, :] = k_t[b, :, h, :].T
            ktm[64:, 1, g, :] = k_c[b, :, h, :].T
            if USE_FP8:
                for t, (qf, kf) in enumerate(((q_t, k_t), (q_c, k_c))):
                    # E-index e -> partition 32*t + e%32, sub-row e//32
                    qT = qf[b, HALF:, h, :].T.reshape(2, 32, HALF)
                    kT = kf[b, :, h, :].T.reshape(2, 32, L)
                    qt8m[32 * t:32 * (t + 1), g] = qT.transpose(1, 0, 2)
                    kt8m[32 * t:32 * (t + 1), g] = kT.transpose(1, 0, 2)
            for t, v_full in enumerate((v_t, v_c)):
                a = 2 * g + t
                vem[:, a, :, :D] = (
                    v_full[b, :, h, :].reshape(NKT, P, D).transpose(1, 0, 2)
                )
                vem[:, a, :, D] = 1.0
        m = {
            "qt": np.ascontiguousarray(qtm).astype(bf16),
            "kt": np.ascontiguousarray(ktm).astype(bf16),
            "ve": np.ascontiguousarray(vem).astype(bf16),
        }
        if USE_FP8:
            m["qt8"] = np.ascontiguousarray(qt8m).astype(fp8)
            m["kt8"] = np.ascontiguousarray(kt8m).astype(fp8)
        in_maps.append(m)
    return in_maps


def _run(in_maps, trace=False):
    if "nc" not in _CACHE:
        _CACHE["nc"] = _build_nc()
    return run_bass_kernel_spmd(
        _CACHE["nc"], in_maps, core_ids=list(range(NCORES)), trace=trace
    )


def kernel(**inputs):
    in_maps = _host_shard(inputs)
    res = _run(in_maps, trace=False)
    v_time = np.empty((B, L, H, D), np.float32)
    v_chan = np.empty((B, L, H, D), np.float32)
    for c in range(NCORES):
        o = np.asarray(res.results[c]["out"]).astype(np.float32)
        # [P, PAIRS, NPASS, 2, 4*DP1] -> [P, PAIRS, 2, NKT, DP1]
        o = o.reshape(P, PAIRS, NPASS, 2, 4, DP1)
        o = o.transpose(1, 3, 0, 2, 4, 5).reshape(PAIRS, 2, P, NKT, DP1)
        o = o[..., :D] / o[..., D:DP1]  # host-side softmax normalization
        for g in range(PAIRS):
            p = PAIRS * c + g
            b, h = divmod(p, H)
            # q = 128*j + qq lives at o[g, t, qq, j, :]
            v_time[b, :, h, :] = o[g, 0].transpose(1, 0, 2).reshape(L, D)
            v_chan[b, :, h, :] = o[g, 1].transpose(1, 0, 2).reshape(L, D)
    return v_time, v_chan


# revision 42
# speedup vs baseline: 1.2554x; 1.0332x over previous
"""AnomalyAttention (two causal attentions per (b,h)) on 8 TRN2 NeuronCores.

Sharding: B*H = 16 (batch, head) pairs -> 2 pairs per core. Each core runs
4 independent causal attentions (time + channel for each of its 2 pairs).
No cross-core communication.

v2 layout ("transposed PV", quarter passes, dual-engine exp, host norm):
  - Queries processed in 4 quarter-passes of 512 per pair (PSUM budget:
    3x2 banks of score tiles in flight + 2 banks of PV accumulators).
  - Per (pass, key-tile) chunk both attention types share one score tile
    [128, 1024]: t0 at cols [0:w], t1 at [512:512+w]. One exp instruction
    covers both halves (merged) when the gap is small enough.
  - exp is split ~50/50 between ACT (exact table exp, bf16 out) and DVE
    (single-op Schraudolph: tensor_scalar fp32 -> int16 computing the bf16
    BIT PATTERN of exp directly; ~3.3% elementwise, cancels in the
    numerator/denominator ratio). Static greedy balance at build time.
  - Diagonal blocks masked post-exp by GPSIMD affine_select (zero fill).
  - PV: out[q, 65] += P^T.T @ V_ext per (j, k); V_ext col 64 is ones so
    col 64 accumulates the softmax denominator.
  - NO device epilogue: raw [128, 4, 65] accumulators are copied
    PSUM->SBUF as bf16 (copy instruction also greedy ACT/DVE) and DMA'd
    out; the host does out[..., :64] / out[..., 64:65].
PSUM discipline: start=True clears has_written for the WHOLE bank, so
exactly one start per bank (first matmul emitted into it).
"""

import math
from contextlib import ExitStack

import ml_dtypes
import numpy as np

import concourse.bacc as bacc
import concourse.mybir as mybir
import concourse.tile as tile
from concourse.bass_utils import run_bass_kernel_spmd

B, L, H, E, D = 2, 2048, 8, 64, 64
NCORES = 8
PAIRS = (B * H) // NCORES          # (b,h) pairs per core = 2
NATT = 2 * PAIRS                   # attentions per core = 4
SCALE = 1.0 / math.sqrt(E)
P = 128                            # partitions / key-tile size
NKT = L // P                       # 16 key tiles
QP = 512                           # quarter-pass query width
NPASS = L // QP                    # 4 passes per pair
DP1 = D + 1                        # value cols + denominator ones-column
F32 = mybir.dt.float32
I16 = mybir.dt.int16
BF16 = mybir.dt.bfloat16

LOG2E = 1.4426950408889634
# int16 Schraudolph: bf16 bits of exp(SCALE*s) = round(s*SCH_A + SCH_B)
SCH_C = 5.6
SCH_A = SCALE * LOG2E * (1 << 7)
SCH_B = float((127 << 7) - SCH_C)

HALF = L // 2
# fp8 DoubleRow scores for passes 2-3 (queries >= 1024): those rows average
# over >=1024 softmax terms, so the ~e4m3 quantization noise on Q/K washes
# out (measured ~6e-3 max-normalized); early rows keep bf16 exactness.
USE_FP8 = False
FP8 = mybir.dt.float8e4

_CACHE = {}

# engine cost models (ns) for the static greedy exp balance; for aligned
# (gapped) chunks the exp either covers the gap (merged) or splits in two
def _act_cost(w, toff):
    if toff == w:
        return (2 * w + 352) / 1.2, True
    merged = (toff + w + 352) / 1.2
    split = (2 * w + 704) / 1.2
    return (merged, True) if merged <= split else (split, False)


def _dve_cost(w, toff):
    if toff == w:
        return 2 * w * 1.04 + 150, True
    merged = (toff + w) * 1.04 + 150
    split = 2 * w * 1.04 + 300
    return (merged, True) if merged <= split else (split, False)


def _build_nc():
    nc = bacc.Bacc()
    qt = nc.declare_dram_parameter("qt", [P, PAIRS, L], BF16, isOutput=False)
    # kt zero-padded to full 128 contraction rows per type: rows 0-63 hold
    # kt_time (t=0) / zeros (t=1), rows 64-127 zeros / kt_chan. This lets the
    # score matmul's MOVING operand (qt) span all 128 partitions -> full SBUF
    # port bandwidth. The extra zero contraction rows are free: matmul cost
    # depends only on output columns.
    kt = nc.declare_dram_parameter("kt", [P, 2, PAIRS, L], BF16, isOutput=False)
    if USE_FP8:
        # E-dim folded as [32 partitions, 2 sub-rows] for DoubleRow
        qt8 = nc.declare_dram_parameter("qt8", [64, PAIRS, 2, HALF], FP8, isOutput=False)
        kt8 = nc.declare_dram_parameter("kt8", [64, PAIRS, 2, L], FP8, isOutput=False)
    else:
        qt8 = kt8 = None
    ve = nc.declare_dram_parameter("ve", [P, NATT, NKT, DP1], BF16, isOutput=False)
    # raw accumulators, partition-major so output DMAs are dim-aligned:
    # out[qq, g, pss, t, jj*DP1+d] = PV accum for attention 2g+t, query
    # 128*(4*pss+jj)+qq, value-col d (d=64 is the softmax denominator)
    out = nc.declare_dram_parameter(
        "out", [P, PAIRS, NPASS, 2, 4 * DP1], BF16, isOutput=True
    )

    with tile.TileContext(nc) as tc:
        with ExitStack() as ctx:
            _body(ctx, tc, qt, kt, qt8, kt8, ve, out)
    nc.finalize()
    return nc


def _body(ctx, tc, qt, kt, qt8, kt8, ve, out):
    nc = tc.nc
    Exp = mybir.ActivationFunctionType.Exp
    Copy = mybir.ActivationFunctionType.Copy

    persist = ctx.enter_context(tc.tile_pool(name="persist", bufs=1))
    s_pool = ctx.enter_context(tc.tile_pool(name="s_pool", bufs=3, space="PSUM"))
    pv_pool = ctx.enter_context(tc.tile_pool(name="pv_pool", bufs=1, space="PSUM"))
    p_pool = ctx.enter_context(tc.tile_pool(name="p_pool", bufs=4))
    ob_pool = ctx.enter_context(tc.tile_pool(name="ob_pool", bufs=3))

    qt_sb = persist.tile([P, PAIRS, L], BF16)
    kt_sb = persist.tile([P, 2, PAIRS, L], BF16)
    ve_sb = persist.tile([P, NATT, NKT, DP1], BF16)
    if USE_FP8:
        qt8_sb = persist.tile([64, PAIRS, 2, HALF], FP8)
        kt8_sb = persist.tile([64, PAIRS, 2, L], FP8)

    # staged input DMA in need-order across three trigger queues, so each
    # pass's operands land just before the PE reaches them
    nc.gpsimd.dma_start(out=kt_sb[:, :, 0, 0:P], in_=kt[:, :, 0, 0:P])
    nc.scalar.dma_start(out=qt_sb[:, 0, 0:QP], in_=qt[:, 0, 0:QP])
    nc.sync.dma_start(out=kt_sb[:, :, 0, P:QP], in_=kt[:, :, 0, P:QP])
    nc.gpsimd.dma_start(out=ve_sb[:, 0], in_=ve[:, 0])
    nc.gpsimd.dma_start(out=ve_sb[:, 1], in_=ve[:, 1])
    nc.scalar.dma_start(out=qt_sb[:, 0, QP:L], in_=qt[:, 0, QP:L])
    nc.sync.dma_start(out=kt_sb[:, :, 0, QP:2 * QP], in_=kt[:, :, 0, QP:2 * QP])
    nc.sync.dma_start(out=kt_sb[:, :, 0, 2 * QP:L], in_=kt[:, :, 0, 2 * QP:L])
    if USE_FP8:
        nc.scalar.dma_start(out=qt8_sb[:, 0], in_=qt8[:, 0])
        nc.sync.dma_start(out=kt8_sb[:, 0], in_=kt8[:, 0])
    nc.sync.dma_start(out=kt_sb[:, :, 1], in_=kt[:, :, 1])
    nc.scalar.dma_start(out=qt_sb[:, 1], in_=qt[:, 1])
    nc.gpsimd.dma_start(out=ve_sb[:, 2], in_=ve[:, 2])
    nc.gpsimd.dma_start(out=ve_sb[:, 3], in_=ve[:, 3])
    if USE_FP8:
        nc.scalar.dma_start(out=qt8_sb[:, 1], in_=qt8[:, 1])
        nc.sync.dma_start(out=kt8_sb[:, 1], in_=kt8[:, 1])

    # warm the ACT exp table (emitted after the DMA triggers so those fire
    # first; the table load then hides under the input transfers)
    warm = persist.tile([1, 8], F32)
    nc.vector.memset(warm, 0.0)
    nc.scalar.activation(warm, warm, Exp)

    # warm the PE HAM clock gate during the input-DMA wait: ~6us of dummy
    # matmuls flip the PE to 2.4GHz before real work arrives (the activity
    # monitor needs ~3.4us of sustained busy; idle gaps < 3.4us keep it warm)
    dm = persist.tile([P, 256], BF16)
    nc.vector.memset(dm, 0.0)
    wps = s_pool.tile([P, 2 * QP], F32, tag="s", name="warm_s")
    for _ in range(40):
        nc.tensor.matmul(
            wps[:, 0:256], lhsT=dm[:, 0:P], rhs=dm,
            start=True, stop=True, skip_group_check=True,
        )

    # static greedy balance state: accumulated busy ns per exp engine
    acc = {"act": 0.0, "dve": 0.0}

    def pick_engine(cost_act, cost_dve):
        # assign to the engine minimizing the resulting makespan
        if max(acc["act"] + cost_act, acc["dve"]) <= max(
            acc["dve"] + cost_dve, acc["act"]
        ):
            acc["act"] += cost_act
            return "act"
        acc["dve"] += cost_dve
        return "dve"

    def emit_exp(s_t, w, toff, diag, force_act=False):
        """exp both halves of a combined score tile [t0: 0..w, t1:
        toff..toff+w]; merged into one instruction when the gap is small.
        Returns pT (bf16 view)."""
        ca, ma = _act_cost(w, toff)
        cv, mv = _dve_cost(w, toff)
        if force_act:
            acc["act"] += ca
            eng = "act"
        else:
            eng = pick_engine(ca, cv)
        merged = ma if eng == "act" else mv
        ranges = [(0, toff + w)] if merged else [(0, w), (toff, toff + w)]
        if eng == "act":
            pT = p_pool.tile([P, 2 * QP], BF16, tag="pa", name="pa")
            for lo, hi in ranges:
                nc.scalar.activation(pT[:, lo:hi], s_t[:, lo:hi], Exp, scale=SCALE)
            pTb = pT
        else:
            pTi = p_pool.tile([P, 2 * QP], I16, tag="pi", name="pi")
            for lo, hi in ranges:
                nc.vector.tensor_scalar(
                    out=pTi[:, lo:hi], in0=s_t[:, lo:hi],
                    scalar1=float(SCH_A), scalar2=float(SCH_B),
                    op0=mybir.AluOpType.mult, op1=mybir.AluOpType.add,
                )
            pTb = pTi.bitcast(BF16)
        if diag:
            # zero the strictly-upper triangle of the diagonal 128-block
            # (query < key) for both types
            for lo in (0, toff):
                nc.gpsimd.affine_select(
                    out=pTb[:, lo:lo + P], in_=pTb[:, lo:lo + P],
                    compare_op=mybir.AluOpType.is_ge, fill=0.0,
                    base=0, channel_multiplier=-1, pattern=[[1, P]],
                )
        return pTb

    def emit_copy(pv, g, pss, jlo, jhi, last):
        """pv accumulators for j-tiles [jlo, jhi) of this pass -> SBUF bf16
        -> DRAM (incl denominator col 64; the host divides)."""
        n = (jhi - jlo) * DP1
        ob = ob_pool.tile([P, 2, n], BF16, tag=f"ob{n}", name="ob")
        pv_v = pv[:, :, jlo * DP1:jhi * DP1]
        cost_a = (2 * n + 352) / 1.2
        cost_v = 2 * n * 1.04 + 150
        if pick_engine(cost_a, cost_v) == "act":
            nc.scalar.activation(ob, pv_v, Copy)
        else:
            nc.vector.tensor_copy(out=ob, in_=pv_v)
        eng = nc.scalar if last else (nc.sync if pss % 2 == 0 else nc.scalar)
        eng.dma_start(
            out=out[:, g, pss, :, jlo * DP1:jhi * DP1], in_=ob
        )

    # one flat, globally-pipelined schedule over all (pair, pass, key-tile)
    # chunks: PV matmuls trail two chunks behind their exp (so PE
    # instructions enter the queue with satisfied deps) and are interleaved
    # between the next chunk's score matmuls so each PV LDWEIGHTS can
    # prefetch under a long score stream.
    class Pass:
        def __init__(self, g, pss):
            self.g, self.pss = g, pss
            self.q0, self.j0 = pss * QP, 4 * pss
            self.kmax = 4 * (pss + 1)
            # one PV accumulator tile for both types: t0 in bank 0 (cols
            # 0:260), t1 in bank 1 (cols 512:772)
            self.pv = pv_pool.tile([P, 2, QP], F32, tag="pv", name="pv")
            self.started = [False, False]

        def emit_pv(self, t, k, qlo, w, toff, pTb):
            a = 2 * self.g + t
            for j in range(max(self.j0, k), self.j0 + 4):
                col = P * j - qlo + t * toff
                first = not self.started[t]
                self.started[t] = True
                nc.tensor.matmul(
                    self.pv[:, t, (j - self.j0) * DP1:(j - self.j0 + 1) * DP1],
                    lhsT=pTb[:, col:col + P],
                    rhs=ve_sb[:, a, k, :],
                    start=first,
                    stop=(k == j),
                    skip_group_check=True,
                )

    chunks = []
    for g in range(PAIRS):
        for pss in range(NPASS):
            for k in range(4 * (pss + 1)):
                chunks.append((g, pss, k))

    pend = []
    cur_pass = None

    def pop_pv(half):
        """Emit one type's trailing PV matmuls (half 0 -> t0, 1 -> t1 and
        retire); called between score segments of the current chunk."""
        if not pend:
            return
        pa, kk, qq, ww, to, pp, done = pend[0]
        pa.emit_pv(half, kk, qq, ww, to, pp)
        if half == 1:
            pend.pop(0)
            last = pa.g == PAIRS - 1 and pa.pss == NPASS - 1
            if last and kk == pa.kmax - 2:
                # final pass: j-tiles 0..2 are already complete after the
                # second-to-last key-tile; flush them early so only a tiny
                # copy+DMA trails the very last matmul
                emit_copy(pa.pv, pa.g, pa.pss, 0, 3, False)
            if done:
                # final key-tile of its pass: accumulators complete, flush
                if last:
                    emit_copy(pa.pv, pa.g, pa.pss, 3, 4, True)
                else:
                    emit_copy(pa.pv, pa.g, pa.pss, 0, 4, False)

    for g, pss, k in chunks:
        if cur_pass is None or (cur_pass.g, cur_pass.pss) != (g, pss):
            cur_pass = Pass(g, pss)
        q0 = pss * QP
        qlo = max(q0, P * k)
        w = q0 + QP - qlo
        diag = qlo == P * k
        s_t = s_pool.tile([P, 2 * QP], F32, tag="s", name="s")
        fp8 = USE_FP8 and pss >= 2
        # bf16 chunks pack gap-free: t0 at [0:w], t1 at [w:2w], with matmul
        # outputs split at absolute 512-col PSUM bank boundaries (first
        # matmul into each bank carries start=True, which clears the whole
        # bank's has_written; followers use False). fp8 DoubleRow matmuls
        # fault on such start=False continuation segments, so fp8 chunks use
        # 512-aligned placement (t1 at [512:512+w], a gap for partial widths).
        toff = QP if fp8 else w
        if fp8:
            segs = [(0, 0, w, True), (1, QP, QP + w, True)]
        else:
            segs = [(0, 0, w, True)]
            lo = w
            while lo < 2 * w:
                hi = min(2 * w, (lo // QP + 1) * QP)
                segs.append((1, lo, hi, lo % QP == 0))
                lo = hi
        emitted_pv = 0
        for t, lo, hi, bank_first in segs:
            c0 = qlo + lo - t * toff
            c1 = qlo + hi - t * toff
            if fp8:
                nc.tensor.matmul(
                    s_t[:, lo:hi],
                    lhsT=kt8_sb[32 * t:32 * (t + 1), g, :, P * k:P * (k + 1)],
                    rhs=qt8_sb[32 * t:32 * (t + 1), g, :, c0 - HALF:c1 - HALF],
                    start=bank_first,
                    stop=True,
                    perf_mode=mybir.MatmulPerfMode.DoubleRow,
                    skip_group_check=True,
                )
            else:
                nc.tensor.matmul(
                    s_t[:, lo:hi],
                    lhsT=kt_sb[:, t, g, P * k:P * (k + 1)],
                    rhs=qt_sb[:, g, c0:c1],
                    start=bank_first,
                    stop=True,
                    skip_group_check=True,
                )
            # interleave the trailing PV work between score segments
            if len(pend) == 3 and emitted_pv < 2:
                pop_pv(emitted_pv)
                emitted_pv += 1
        while len(pend) == 3 and emitted_pv < 2:
            pop_pv(emitted_pv)
            emitted_pv += 1
        # queries 0:511 average over few softmax terms, so the Schraudolph
        # ~3% element error would not cancel there: keep the first key-tile
        # of pass 0 on the exact ACT exp
        pTb = emit_exp(s_t, w, toff, diag, force_act=(pss == 0 and k == 0))
        pend.append(
            (cur_pass, k, qlo, w, toff, pTb, k == cur_pass.kmax - 1)
        )
    while pend:
        pop_pv(0)
        pop_pv(1)


def _host_shard(inputs):
    """Build the 8 per-core input maps from full inputs (host-side numpy)."""
    q_t = np.asarray(inputs["queries_time"], dtype=np.float32)
    k_t = np.asarray(inputs["keys_time"], dtype=np.float32)
    v_t = np.asarray(inputs["values_time"], dtype=np.float32)
    q_c = np.asarray(inputs["queries_channel"], dtype=np.float32)
    k_c = np.asarray(inputs["keys_channel"], dtype=np.float32)
    v_c = np.asarray(inputs["values_channel"], dtype=np.float32)

    bf16 = ml_dtypes.bfloat16
    fp8 = ml_dtypes.float8_e4m3
    in_maps = []
    for c in range(NCORES):
        vem = np.empty((P, NATT, NKT, DP1), np.float32)
        qtm = np.empty((P, PAIRS, L), np.float32)
        ktm = np.zeros((P, 2, PAIRS, L), np.float32)
        qt8m = np.empty((64, PAIRS, 2, HALF), np.float32)
        kt8m = np.empty((64, PAIRS, 2, L), np.float32)
        for g in range(PAIRS):
            p = PAIRS * c + g
            b, h = divmod(p, H)
            qtm[:64, g, :] = q_t[b, :, h, :].T
            qtm[64:, g, :] = q_c[b, :, h, :].T
            ktm[:64, 0, g, :] = k_t[b, :, h, :].T
            ktm[64:, 1, g, :] = k_c[b, :, h, :].T
            if USE_FP8:
                for t, (qf, kf) in enumerate(((q_t, k_t), (q_c, k_c))):
                    # E-index e -> partition 32*t + e%32, sub-row e//32
                    qT = qf[b, HALF:, h, :].T.reshape(2, 32, HALF)
                    kT = kf[b, :, h, :].T.reshape(2, 32, L)
                    qt8m[32 * t:32 * (t + 1), g] = qT.transpose(1, 0, 2)
                    kt8m[32 * t:32 * (t + 1), g] = kT.transpose(1, 0, 2)
            for t, v_full in enumerate((v_t, v_c)):
                a = 2 * g + t
                vem[:, a, :, :D] = (
                    v_full[b, :, h, :].reshape(NKT, P, D).transpose(1, 0, 2)
                )
                vem[:, a, :, D] = 1.0
        m = {
            "qt": np.ascontiguousarray(qtm).astype(bf16),
            "kt": np.ascontiguousarray(ktm).astype(bf16),
            "ve": np.ascontiguousarray(vem).astype(bf16),
        }
        if USE_FP8:
            m["qt8"] = np.ascontiguousarray(qt8m).astype(fp8)
            m["kt8"] = np.ascontiguousarray(kt8m).astype(fp8)
        in_maps.append(m)
    return in_maps


def _run(in_maps, trace=False):
    if "nc" not in _CACHE:
        _CACHE["nc"] = _build_nc()
    return run_bass_kernel_spmd(
        _CACHE["nc"], in_maps, core_ids=list(range(NCORES)), trace=trace
    )


def kernel(**inputs):
    in_maps = _host_shard(inputs)
    res = _run(in_maps, trace=False)
    v_time = np.empty((B, L, H, D), np.float32)
    v_chan = np.empty((B, L, H, D), np.float32)
    for c in range(NCORES):
        o = np.asarray(res.results[c]["out"]).astype(np.float32)
        # [P, PAIRS, NPASS, 2, 4*DP1] -> [P, PAIRS, 2, NKT, DP1]
        o = o.reshape(P, PAIRS, NPASS, 2, 4, DP1)
        o = o.transpose(1, 3, 0, 2, 4, 5).reshape(PAIRS, 2, P, NKT, DP1)
        o = o[..., :D] / o[..., D:DP1]  # host-side softmax normalization
        for g in range(PAIRS):
            p = PAIRS * c + g
            b, h = divmod(p, H)
            # q = 128*j + qq lives at o[g, t, qq, j, :]
            v_time[b, :, h, :] = o[g, 0].transpose(1, 0, 2).reshape(L, D)
            v_chan[b, :, h, :] = o[g, 1].transpose(1, 0, 2).reshape(L, D)
    return v_time, v_chan


# revision 44
# speedup vs baseline: 1.3494x; 1.0749x over previous
"""AnomalyAttention (two causal attentions per (b,h)) on 8 TRN2 NeuronCores.

Sharding: B*H = 16 (batch, head) pairs -> 2 pairs per core. Each core runs
4 independent causal attentions (time + channel for each of its 2 pairs).
No cross-core communication.

v2 layout ("transposed PV", quarter passes, dual-engine exp, host norm):
  - Queries processed in 4 quarter-passes of 512 per pair (PSUM budget:
    3x2 banks of score tiles in flight + 2 banks of PV accumulators).
  - Per (pass, key-tile) chunk both attention types share one score tile
    [128, 1024]: t0 at cols [0:w], t1 at [512:512+w]. One exp instruction
    covers both halves (merged) when the gap is small enough.
  - exp is split ~50/50 between ACT (exact table exp, bf16 out) and DVE
    (single-op Schraudolph: tensor_scalar fp32 -> int16 computing the bf16
    BIT PATTERN of exp directly; ~3.3% elementwise, cancels in the
    numerator/denominator ratio). Static greedy balance at build time.
  - Diagonal blocks masked post-exp by GPSIMD affine_select (zero fill).
  - PV: out[q, 65] += P^T.T @ V_ext per (j, k); V_ext col 64 is ones so
    col 64 accumulates the softmax denominator.
  - NO device epilogue: raw [128, 4, 65] accumulators are copied
    PSUM->SBUF as bf16 (copy instruction also greedy ACT/DVE) and DMA'd
    out; the host does out[..., :64] / out[..., 64:65].
PSUM discipline: start=True clears has_written for the WHOLE bank, so
exactly one start per bank (first matmul emitted into it).
"""

import math
from contextlib import ExitStack

import ml_dtypes
import numpy as np

import concourse.bacc as bacc
import concourse.mybir as mybir
import concourse.tile as tile
from concourse.bass_utils import run_bass_kernel_spmd

B, L, H, E, D = 2, 2048, 8, 64, 64
NCORES = 8
PAIRS = (B * H) // NCORES          # (b,h) pairs per core = 2
NATT = 2 * PAIRS                   # attentions per core = 4
SCALE = 1.0 / math.sqrt(E)
P = 128                            # partitions / key-tile size
NKT = L // P                       # 16 key tiles
QP = 512                           # quarter-pass query width
NPASS = L // QP                    # 4 passes per pair
DP1 = D + 1                        # value cols + denominator ones-column
F32 = mybir.dt.float32
I16 = mybir.dt.int16
BF16 = mybir.dt.bfloat16

LOG2E = 1.4426950408889634
# int16 Schraudolph: bf16 bits of exp(SCALE*s) = round(s*SCH_A + SCH_B)
SCH_C = 5.6
SCH_A = SCALE * LOG2E * (1 << 7)
SCH_B = float((127 << 7) - SCH_C)

HALF = L // 2
# fp8 DoubleRow scores for passes 2-3 (queries >= 1024): those rows average
# over >=1024 softmax terms, so the ~e4m3 quantization noise on Q/K washes
# out (measured ~6e-3 max-normalized); early rows keep bf16 exactness.
USE_FP8 = False
FP8 = mybir.dt.float8e4

_CACHE = {}

# engine cost models (ns) for the static greedy exp balance; for aligned
# (gapped) chunks the exp either covers the gap (merged) or splits in two
def _act_cost(w, toff):
    if toff == w:
        return (2 * w + 352) / 1.2, True
    merged = (toff + w + 352) / 1.2
    split = (2 * w + 704) / 1.2
    return (merged, True) if merged <= split else (split, False)


def _dve_cost(w, toff):
    if toff == w:
        return 2 * w * 1.04 + 150, True
    merged = (toff + w) * 1.04 + 150
    split = 2 * w * 1.04 + 300
    return (merged, True) if merged <= split else (split, False)


def _build_nc():
    nc = bacc.Bacc()
    qt = nc.declare_dram_parameter("qt", [P, PAIRS, L], BF16, isOutput=False)
    # kt zero-padded to full 128 contraction rows per type: rows 0-63 hold
    # kt_time (t=0) / zeros (t=1), rows 64-127 zeros / kt_chan. This lets the
    # score matmul's MOVING operand (qt) span all 128 partitions -> full SBUF
    # port bandwidth. The extra zero contraction rows are free: matmul cost
    # depends only on output columns.
    kt = nc.declare_dram_parameter("kt", [P, 2, PAIRS, L], BF16, isOutput=False)
    if USE_FP8:
        # E-dim folded as [32 partitions, 2 sub-rows] for DoubleRow
        qt8 = nc.declare_dram_parameter("qt8", [64, PAIRS, 2, HALF], FP8, isOutput=False)
        kt8 = nc.declare_dram_parameter("kt8", [64, PAIRS, 2, L], FP8, isOutput=False)
    else:
        qt8 = kt8 = None
    ve = nc.declare_dram_parameter("ve", [P, NATT, NKT, DP1], BF16, isOutput=False)
    # raw accumulators, partition-major so output DMAs are dim-aligned:
    # out[qq, g, pss, t, jj*DP1+d] = PV accum for attention 2g+t, query
    # 128*(4*pss+jj)+qq, value-col d (d=64 is the softmax denominator)
    out = nc.declare_dram_parameter(
        "out", [P, PAIRS, NPASS, 2, 4 * DP1], BF16, isOutput=True
    )

    with tile.TileContext(nc) as tc:
        with ExitStack() as ctx:
            _body(ctx, tc, qt, kt, qt8, kt8, ve, out)
    nc.finalize()
    return nc


def _body(ctx, tc, qt, kt, qt8, kt8, ve, out):
    nc = tc.nc
    Exp = mybir.ActivationFunctionType.Exp
    Copy = mybir.ActivationFunctionType.Copy

    persist = ctx.enter_context(tc.tile_pool(name="persist", bufs=1))
    s_pool = ctx.enter_context(tc.tile_pool(name="s_pool", bufs=3, space="PSUM"))
    pv_pool = ctx.enter_context(tc.tile_pool(name="pv_pool", bufs=1, space="PSUM"))
    p_pool = ctx.enter_context(tc.tile_pool(name="p_pool", bufs=5))
    ob_pool = ctx.enter_context(tc.tile_pool(name="ob_pool", bufs=3))

    qt_sb = persist.tile([P, PAIRS, L], BF16)
    kt_sb = persist.tile([P, 2, PAIRS, L], BF16)
    ve_sb = persist.tile([P, NATT, NKT, DP1], BF16)
    if USE_FP8:
        qt8_sb = persist.tile([64, PAIRS, 2, HALF], FP8)
        kt8_sb = persist.tile([64, PAIRS, 2, L], FP8)

    # staged input DMA in need-order across three trigger queues, so each
    # pass's operands land just before the PE reaches them
    nc.gpsimd.dma_start(out=kt_sb[:, :, 0, 0:P], in_=kt[:, :, 0, 0:P])
    nc.scalar.dma_start(out=qt_sb[:, 0, 0:QP], in_=qt[:, 0, 0:QP])
    nc.sync.dma_start(out=kt_sb[:, :, 0, P:QP], in_=kt[:, :, 0, P:QP])
    nc.gpsimd.dma_start(out=ve_sb[:, 0], in_=ve[:, 0])
    nc.gpsimd.dma_start(out=ve_sb[:, 1], in_=ve[:, 1])
    nc.scalar.dma_start(out=qt_sb[:, 0, QP:L], in_=qt[:, 0, QP:L])
    nc.sync.dma_start(out=kt_sb[:, :, 0, QP:2 * QP], in_=kt[:, :, 0, QP:2 * QP])
    nc.sync.dma_start(out=kt_sb[:, :, 0, 2 * QP:L], in_=kt[:, :, 0, 2 * QP:L])
    if USE_FP8:
        nc.scalar.dma_start(out=qt8_sb[:, 0], in_=qt8[:, 0])
        nc.sync.dma_start(out=kt8_sb[:, 0], in_=kt8[:, 0])
    nc.sync.dma_start(out=kt_sb[:, :, 1], in_=kt[:, :, 1])
    nc.scalar.dma_start(out=qt_sb[:, 1], in_=qt[:, 1])
    nc.gpsimd.dma_start(out=ve_sb[:, 2], in_=ve[:, 2])
    nc.gpsimd.dma_start(out=ve_sb[:, 3], in_=ve[:, 3])
    if USE_FP8:
        nc.scalar.dma_start(out=qt8_sb[:, 1], in_=qt8[:, 1])
        nc.sync.dma_start(out=kt8_sb[:, 1], in_=kt8[:, 1])

    # warm the ACT exp table (emitted after the DMA triggers so those fire
    # first; the table load then hides under the input transfers)
    warm = persist.tile([1, 8], F32)
    nc.vector.memset(warm, 0.0)
    nc.scalar.activation(warm, warm, Exp)

    # warm the PE HAM clock gate during the input-DMA wait: ~6us of dummy
    # matmuls flip the PE to 2.4GHz before real work arrives (the activity
    # monitor needs ~3.4us of sustained busy; idle gaps < 3.4us keep it warm)
    dm = persist.tile([P, 256], BF16)
    nc.vector.memset(dm, 0.0)
    wps = s_pool.tile([P, 2 * QP], F32, tag="s", name="warm_s")
    for _ in range(40):
        nc.tensor.matmul(
            wps[:, 0:256], lhsT=dm[:, 0:P], rhs=dm,
            start=True, stop=True, skip_group_check=True,
        )

    # static greedy balance state: accumulated busy ns per exp engine
    acc = {"act": 0.0, "dve": 0.0}

    def pick_engine(cost_act, cost_dve):
        # assign to the engine minimizing the resulting makespan
        if max(acc["act"] + cost_act, acc["dve"]) <= max(
            acc["dve"] + cost_dve, acc["act"]
        ):
            acc["act"] += cost_act
            return "act"
        acc["dve"] += cost_dve
        return "dve"

    def emit_exp(s_t, w, toff, diag, force_act=False):
        """exp both halves of a combined score tile [t0: 0..w, t1:
        toff..toff+w]; merged into one instruction when the gap is small.
        Returns pT (bf16 view)."""
        ca, ma = _act_cost(w, toff)
        cv, mv = _dve_cost(w, toff)
        if force_act:
            acc["act"] += ca
            eng = "act"
        else:
            eng = pick_engine(ca, cv)
        merged = ma if eng == "act" else mv
        ranges = [(0, toff + w)] if merged else [(0, w), (toff, toff + w)]
        if eng == "act":
            pT = p_pool.tile([P, 2 * QP], BF16, tag="pa", name="pa")
            for lo, hi in ranges:
                nc.scalar.activation(pT[:, lo:hi], s_t[:, lo:hi], Exp, scale=SCALE)
            pTb = pT
        else:
            pTi = p_pool.tile([P, 2 * QP], I16, tag="pi", name="pi")
            for lo, hi in ranges:
                nc.vector.tensor_scalar(
                    out=pTi[:, lo:hi], in0=s_t[:, lo:hi],
                    scalar1=float(SCH_A), scalar2=float(SCH_B),
                    op0=mybir.AluOpType.mult, op1=mybir.AluOpType.add,
                )
            pTb = pTi.bitcast(BF16)
        if diag:
            # zero the strictly-upper triangle of the diagonal 128-block
            # (query < key) for both types
            for lo in (0, toff):
                nc.gpsimd.affine_select(
                    out=pTb[:, lo:lo + P], in_=pTb[:, lo:lo + P],
                    compare_op=mybir.AluOpType.is_ge, fill=0.0,
                    base=0, channel_multiplier=-1, pattern=[[1, P]],
                )
        return pTb

    def emit_copy(pv, g, pss, jlo, jhi, last):
        """pv accumulators for j-tiles [jlo, jhi) of this pass -> SBUF bf16
        -> DRAM (incl denominator col 64; the host divides)."""
        n = (jhi - jlo) * DP1
        ob = ob_pool.tile([P, 2, n], BF16, tag=f"ob{n}", name="ob")
        pv_v = pv[:, :, jlo * DP1:jhi * DP1]
        cost_a = (2 * n + 352) / 1.2
        cost_v = 2 * n * 1.04 + 150
        if pick_engine(cost_a, cost_v) == "act":
            nc.scalar.activation(ob, pv_v, Copy)
        else:
            nc.vector.tensor_copy(out=ob, in_=pv_v)
        eng = nc.scalar if last else (nc.sync if pss % 2 == 0 else nc.scalar)
        eng.dma_start(
            out=out[:, g, pss, :, jlo * DP1:jhi * DP1], in_=ob
        )

    # one flat, globally-pipelined schedule over all (pair, pass, key-tile)
    # chunks: PV matmuls trail two chunks behind their exp (so PE
    # instructions enter the queue with satisfied deps) and are interleaved
    # between the next chunk's score matmuls so each PV LDWEIGHTS can
    # prefetch under a long score stream.
    class Pass:
        def __init__(self, g, pss):
            self.g, self.pss = g, pss
            self.q0, self.j0 = pss * QP, 4 * pss
            self.kmax = 4 * (pss + 1)
            # one PV accumulator tile for both types: t0 in bank 0 (cols
            # 0:260), t1 in bank 1 (cols 512:772)
            self.pv = pv_pool.tile([P, 2, QP], F32, tag="pv", name="pv")
            self.started = [False, False]

        def emit_pv(self, t, k, qlo, w, toff, pTb):
            a = 2 * self.g + t
            js = list(range(max(self.j0, k), self.j0 + 4))
            if js and js[0] == k:
                # the j==k matmul reads the affine_select-masked diag block;
                # emit it LAST so the independent j>k matmuls are not stuck
                # behind its GPSIMD dependency in the PE FIFO
                js = js[1:] + [js[0]]
            for j in js:
                col = P * j - qlo + t * toff
                first = not self.started[t]
                self.started[t] = True
                nc.tensor.matmul(
                    self.pv[:, t, (j - self.j0) * DP1:(j - self.j0 + 1) * DP1],
                    lhsT=pTb[:, col:col + P],
                    rhs=ve_sb[:, a, k, :],
                    start=first,
                    stop=(k == j),
                    skip_group_check=True,
                )

    chunks = []
    for g in range(PAIRS):
        for pss in range(NPASS):
            for k in range(4 * (pss + 1)):
                chunks.append((g, pss, k))

    pend = []
    cur_pass = None

    def pop_pv(half):
        """Emit one type's trailing PV matmuls (half 0 -> t0, 1 -> t1 and
        retire); called between score segments of the current chunk."""
        if not pend:
            return
        pa, kk, qq, ww, to, pp, done = pend[0]
        pa.emit_pv(half, kk, qq, ww, to, pp)
        if half == 1:
            pend.pop(0)
            last = pa.g == PAIRS - 1 and pa.pss == NPASS - 1
            if last and kk == pa.kmax - 2:
                # final pass: j-tiles 0..2 are already complete after the
                # second-to-last key-tile; flush them early so only a tiny
                # copy+DMA trails the very last matmul
                emit_copy(pa.pv, pa.g, pa.pss, 0, 3, False)
            if done:
                # final key-tile of its pass: accumulators complete, flush
                if last:
                    emit_copy(pa.pv, pa.g, pa.pss, 3, 4, True)
                else:
                    emit_copy(pa.pv, pa.g, pa.pss, 0, 4, False)

    for g, pss, k in chunks:
        if cur_pass is None or (cur_pass.g, cur_pass.pss) != (g, pss):
            cur_pass = Pass(g, pss)
        q0 = pss * QP
        qlo = max(q0, P * k)
        w = q0 + QP - qlo
        diag = qlo == P * k
        s_t = s_pool.tile([P, 2 * QP], F32, tag="s", name="s")
        fp8 = USE_FP8 and pss >= 2
        # bf16 chunks pack gap-free: t0 at [0:w], t1 at [w:2w], with matmul
        # outputs split at absolute 512-col PSUM bank boundaries (first
        # matmul into each bank carries start=True, which clears the whole
        # bank's has_written; followers use False). fp8 DoubleRow matmuls
        # fault on such start=False continuation segments, so fp8 chunks use
        # 512-aligned placement (t1 at [512:512+w], a gap for partial widths).
        toff = QP if fp8 else w
        if fp8:
            segs = [(0, 0, w, True), (1, QP, QP + w, True)]
        else:
            segs = [(0, 0, w, True)]
            lo = w
            while lo < 2 * w:
                hi = min(2 * w, (lo // QP + 1) * QP)
                segs.append((1, lo, hi, lo % QP == 0))
                lo = hi
        emitted_pv = 0
        for t, lo, hi, bank_first in segs:
            c0 = qlo + lo - t * toff
            c1 = qlo + hi - t * toff
            if fp8:
                nc.tensor.matmul(
                    s_t[:, lo:hi],
                    lhsT=kt8_sb[32 * t:32 * (t + 1), g, :, P * k:P * (k + 1)],
                    rhs=qt8_sb[32 * t:32 * (t + 1), g, :, c0 - HALF:c1 - HALF],
                    start=bank_first,
                    stop=True,
                    perf_mode=mybir.MatmulPerfMode.DoubleRow,
                    skip_group_check=True,
                )
            else:
                nc.tensor.matmul(
                    s_t[:, lo:hi],
                    lhsT=kt_sb[:, t, g, P * k:P * (k + 1)],
                    rhs=qt_sb[:, g, c0:c1],
                    start=bank_first,
                    stop=True,
                    skip_group_check=True,
                )
            # interleave the trailing PV work between score segments
            if len(pend) == 4 and emitted_pv < 2:
                pop_pv(emitted_pv)
                emitted_pv += 1
        while len(pend) == 4 and emitted_pv < 2:
            pop_pv(emitted_pv)
            emitted_pv += 1
        # queries 0:511 average over few softmax terms, so the Schraudolph
        # ~3% element error would not cancel there: keep the first key-tile
        # of pass 0 on the exact ACT exp
        pTb = emit_exp(s_t, w, toff, diag, force_act=(pss == 0 and k == 0))
        pend.append(
            (cur_pass, k, qlo, w, toff, pTb, k == cur_pass.kmax - 1)
        )
    while pend:
        pop_pv(0)
        pop_pv(1)


def _host_shard(inputs):
    """Build the 8 per-core input maps from full inputs (host-side numpy)."""
    q_t = np.asarray(inputs["queries_time"], dtype=np.float32)
    k_t = np.asarray(inputs["keys_time"], dtype=np.float32)
    v_t = np.asarray(inputs["values_time"], dtype=np.float32)
    q_c = np.asarray(inputs["queries_channel"], dtype=np.float32)
    k_c = np.asarray(inputs["keys_channel"], dtype=np.float32)
    v_c = np.asarray(inputs["values_channel"], dtype=np.float32)

    bf16 = ml_dtypes.bfloat16
    fp8 = ml_dtypes.float8_e4m3
    in_maps = []
    for c in range(NCORES):
        vem = np.empty((P, NATT, NKT, DP1), np.float32)
        qtm = np.empty((P, PAIRS, L), np.float32)
        ktm = np.zeros((P, 2, PAIRS, L), np.float32)
        qt8m = np.empty((64, PAIRS, 2, HALF), np.float32)
        kt8m = np.empty((64, PAIRS, 2, L), np.float32)
        for g in range(PAIRS):
            p = PAIRS * c + g
            b, h = divmod(p, H)
            qtm[:64, g, :] = q_t[b, :, h, :].T
            qtm[64:, g, :] = q_c[b, :, h, :].T
            ktm[:64, 0, g, :] = k_t[b, :, h, :].T
            ktm[64:, 1, g, :] = k_c[b, :, h, :].T
            if USE_FP8:
                for t, (qf, kf) in enumerate(((q_t, k_t), (q_c, k_c))):
                    # E-index e -> partition 32*t + e%32, sub-row e//32
                    qT = qf[b, HALF:, h, :].T.reshape(2, 32, HALF)
                    kT = kf[b, :, h, :].T.reshape(2, 32, L)
                    qt8m[32 * t:32 * (t + 1), g] = qT.transpose(1, 0, 2)
                    kt8m[32 * t:32 * (t + 1), g] = kT.transpose(1, 0, 2)
            for t, v_full in enumerate((v_t, v_c)):
                a = 2 * g + t
                vem[:, a, :, :D] = (
                    v_full[b, :, h, :].reshape(NKT, P, D).transpose(1, 0, 2)
                )
                vem[:, a, :, D] = 1.0
        m = {
            "qt": np.ascontiguousarray(qtm).astype(bf16),
            "kt": np.ascontiguousarray(ktm).astype(bf16),
            "ve": np.ascontiguousarray(vem).astype(bf16),
        }
        if USE_FP8:
            m["qt8"] = np.ascontiguousarray(qt8m).astype(fp8)
            m["kt8"] = np.ascontiguousarray(kt8m).astype(fp8)
        in_maps.append(m)
    return in_maps


def _run(in_maps, trace=False):
    if "nc" not in _CACHE:
        _CACHE["nc"] = _build_nc()
    return run_bass_kernel_spmd(
        _CACHE["nc"], in_maps, core_ids=list(range(NCORES)), trace=trace
    )


def kernel(**inputs):
    in_maps = _host_shard(inputs)
    res = _run(in_maps, trace=False)
    v_time = np.empty((B, L, H, D), np.float32)
    v_chan = np.empty((B, L, H, D), np.float32)
    for c in range(NCORES):
        o = np.asarray(res.results[c]["out"]).astype(np.float32)
        # [P, PAIRS, NPASS, 2, 4*DP1] -> [P, PAIRS, 2, NKT, DP1]
        o = o.reshape(P, PAIRS, NPASS, 2, 4, DP1)
        o = o.transpose(1, 3, 0, 2, 4, 5).reshape(PAIRS, 2, P, NKT, DP1)
        o = o[..., :D] / o[..., D:DP1]  # host-side softmax normalization
        for g in range(PAIRS):
            p = PAIRS * c + g
            b, h = divmod(p, H)
            # q = 128*j + qq lives at o[g, t, qq, j, :]
            v_time[b, :, h, :] = o[g, 0].transpose(1, 0, 2).reshape(L, D)
            v_chan[b, :, h, :] = o[g, 1].transpose(1, 0, 2).reshape(L, D)
    return v_time, v_chan


# revision 45
# speedup vs baseline: 1.3682x; 1.0139x over previous
"""AnomalyAttention (two causal attentions per (b,h)) on 8 TRN2 NeuronCores.

Sharding: B*H = 16 (batch, head) pairs -> 2 pairs per core. Each core runs
4 independent causal attentions (time + channel for each of its 2 pairs).
No cross-core communication.

v2 layout ("transposed PV", quarter passes, dual-engine exp, host norm):
  - Queries processed in 4 quarter-passes of 512 per pair (PSUM budget:
    3x2 banks of score tiles in flight + 2 banks of PV accumulators).
  - Per (pass, key-tile) chunk both attention types share one score tile
    [128, 1024]: t0 at cols [0:w], t1 at [512:512+w]. One exp instruction
    covers both halves (merged) when the gap is small enough.
  - exp is split ~50/50 between ACT (exact table exp, bf16 out) and DVE
    (single-op Schraudolph: tensor_scalar fp32 -> int16 computing the bf16
    BIT PATTERN of exp directly; ~3.3% elementwise, cancels in the
    numerator/denominator ratio). Static greedy balance at build time.
  - Diagonal blocks masked post-exp by GPSIMD affine_select (zero fill).
  - PV: out[q, 65] += P^T.T @ V_ext per (j, k); V_ext col 64 is ones so
    col 64 accumulates the softmax denominator.
  - NO device epilogue: raw [128, 4, 65] accumulators are copied
    PSUM->SBUF as bf16 (copy instruction also greedy ACT/DVE) and DMA'd
    out; the host does out[..., :64] / out[..., 64:65].
PSUM discipline: start=True clears has_written for the WHOLE bank, so
exactly one start per bank (first matmul emitted into it).
"""

import math
from contextlib import ExitStack

import ml_dtypes
import numpy as np

import concourse.bacc as bacc
import concourse.mybir as mybir
import concourse.tile as tile
from concourse.bass_utils import run_bass_kernel_spmd

B, L, H, E, D = 2, 2048, 8, 64, 64
NCORES = 8
PAIRS = (B * H) // NCORES          # (b,h) pairs per core = 2
NATT = 2 * PAIRS                   # attentions per core = 4
SCALE = 1.0 / math.sqrt(E)
P = 128                            # partitions / key-tile size
NKT = L // P                       # 16 key tiles
QP = 512                           # quarter-pass query width
NPASS = L // QP                    # 4 passes per pair
DP1 = D + 1                        # value cols + denominator ones-column
F32 = mybir.dt.float32
I16 = mybir.dt.int16
BF16 = mybir.dt.bfloat16

LOG2E = 1.4426950408889634
# int16 Schraudolph: bf16 bits of exp(SCALE*s) = round(s*SCH_A + SCH_B)
SCH_C = 5.6
SCH_A = SCALE * LOG2E * (1 << 7)
SCH_B = float((127 << 7) - SCH_C)

HALF = L // 2
# fp8 DoubleRow scores for passes 2-3 (queries >= 1024): those rows average
# over >=1024 softmax terms, so the ~e4m3 quantization noise on Q/K washes
# out (measured ~6e-3 max-normalized); early rows keep bf16 exactness.
USE_FP8 = False
FP8 = mybir.dt.float8e4

_CACHE = {}

# engine cost models (ns) for the static greedy exp balance; for aligned
# (gapped) chunks the exp either covers the gap (merged) or splits in two
def _act_cost(w, toff):
    if toff == w:
        return (2 * w + 352) / 1.2, True
    merged = (toff + w + 352) / 1.2
    split = (2 * w + 704) / 1.2
    return (merged, True) if merged <= split else (split, False)


def _dve_cost(w, toff):
    if toff == w:
        return 2 * w * 1.04 + 150, True
    merged = (toff + w) * 1.04 + 150
    split = 2 * w * 1.04 + 300
    return (merged, True) if merged <= split else (split, False)


def _build_nc():
    nc = bacc.Bacc()
    qt = nc.declare_dram_parameter("qt", [P, PAIRS, L], BF16, isOutput=False)
    # kt zero-padded to full 128 contraction rows per type: rows 0-63 hold
    # kt_time (t=0) / zeros (t=1), rows 64-127 zeros / kt_chan. This lets the
    # score matmul's MOVING operand (qt) span all 128 partitions -> full SBUF
    # port bandwidth. The extra zero contraction rows are free: matmul cost
    # depends only on output columns.
    kt = nc.declare_dram_parameter("kt", [P, 2, PAIRS, L], BF16, isOutput=False)
    if USE_FP8:
        # E-dim folded as [32 partitions, 2 sub-rows] for DoubleRow
        qt8 = nc.declare_dram_parameter("qt8", [64, PAIRS, 2, HALF], FP8, isOutput=False)
        kt8 = nc.declare_dram_parameter("kt8", [64, PAIRS, 2, L], FP8, isOutput=False)
    else:
        qt8 = kt8 = None
    ve = nc.declare_dram_parameter("ve", [P, NATT, NKT, DP1], BF16, isOutput=False)
    # raw accumulators, partition-major so output DMAs are dim-aligned:
    # out[qq, g, pss, t, jj*DP1+d] = PV accum for attention 2g+t, query
    # 128*(4*pss+jj)+qq, value-col d (d=64 is the softmax denominator)
    out = nc.declare_dram_parameter(
        "out", [P, PAIRS, NPASS, 2, 4 * DP1], BF16, isOutput=True
    )

    with tile.TileContext(nc) as tc:
        with ExitStack() as ctx:
            _body(ctx, tc, qt, kt, qt8, kt8, ve, out)
    nc.finalize()
    return nc


def _body(ctx, tc, qt, kt, qt8, kt8, ve, out):
    nc = tc.nc
    Exp = mybir.ActivationFunctionType.Exp
    Copy = mybir.ActivationFunctionType.Copy

    persist = ctx.enter_context(tc.tile_pool(name="persist", bufs=1))
    s_pool = ctx.enter_context(tc.tile_pool(name="s_pool", bufs=3, space="PSUM"))
    pv_pool = ctx.enter_context(tc.tile_pool(name="pv_pool", bufs=1, space="PSUM"))
    p_pool = ctx.enter_context(tc.tile_pool(name="p_pool", bufs=6))
    ob_pool = ctx.enter_context(tc.tile_pool(name="ob_pool", bufs=3))

    qt_sb = persist.tile([P, PAIRS, L], BF16)
    kt_sb = persist.tile([P, 2, PAIRS, L], BF16)
    ve_sb = persist.tile([P, NATT, NKT, DP1], BF16)
    if USE_FP8:
        qt8_sb = persist.tile([64, PAIRS, 2, HALF], FP8)
        kt8_sb = persist.tile([64, PAIRS, 2, L], FP8)

    # staged input DMA in need-order across three trigger queues, so each
    # pass's operands land just before the PE reaches them
    nc.gpsimd.dma_start(out=kt_sb[:, :, 0, 0:P], in_=kt[:, :, 0, 0:P])
    nc.scalar.dma_start(out=qt_sb[:, 0, 0:QP], in_=qt[:, 0, 0:QP])
    nc.sync.dma_start(out=kt_sb[:, :, 0, P:QP], in_=kt[:, :, 0, P:QP])
    nc.gpsimd.dma_start(out=ve_sb[:, 0], in_=ve[:, 0])
    nc.gpsimd.dma_start(out=ve_sb[:, 1], in_=ve[:, 1])
    nc.scalar.dma_start(out=qt_sb[:, 0, QP:L], in_=qt[:, 0, QP:L])
    nc.sync.dma_start(out=kt_sb[:, :, 0, QP:2 * QP], in_=kt[:, :, 0, QP:2 * QP])
    nc.sync.dma_start(out=kt_sb[:, :, 0, 2 * QP:L], in_=kt[:, :, 0, 2 * QP:L])
    if USE_FP8:
        nc.scalar.dma_start(out=qt8_sb[:, 0], in_=qt8[:, 0])
        nc.sync.dma_start(out=kt8_sb[:, 0], in_=kt8[:, 0])
    nc.sync.dma_start(out=kt_sb[:, :, 1], in_=kt[:, :, 1])
    nc.scalar.dma_start(out=qt_sb[:, 1], in_=qt[:, 1])
    nc.gpsimd.dma_start(out=ve_sb[:, 2], in_=ve[:, 2])
    nc.gpsimd.dma_start(out=ve_sb[:, 3], in_=ve[:, 3])
    if USE_FP8:
        nc.scalar.dma_start(out=qt8_sb[:, 1], in_=qt8[:, 1])
        nc.sync.dma_start(out=kt8_sb[:, 1], in_=kt8[:, 1])

    # warm the ACT exp table (emitted after the DMA triggers so those fire
    # first; the table load then hides under the input transfers)
    warm = persist.tile([1, 8], F32)
    nc.vector.memset(warm, 0.0)
    nc.scalar.activation(warm, warm, Exp)

    # warm the PE HAM clock gate during the input-DMA wait: ~6us of dummy
    # matmuls flip the PE to 2.4GHz before real work arrives (the activity
    # monitor needs ~3.4us of sustained busy; idle gaps < 3.4us keep it warm)
    dm = persist.tile([P, 256], BF16)
    nc.vector.memset(dm, 0.0)
    wps = s_pool.tile([P, 2 * QP], F32, tag="s", name="warm_s")
    for _ in range(40):
        nc.tensor.matmul(
            wps[:, 0:256], lhsT=dm[:, 0:P], rhs=dm,
            start=True, stop=True, skip_group_check=True,
        )

    # static greedy balance state: accumulated busy ns per exp engine
    acc = {"act": 0.0, "dve": 0.0}

    def pick_engine(cost_act, cost_dve):
        # assign to the engine minimizing the resulting makespan
        if max(acc["act"] + cost_act, acc["dve"]) <= max(
            acc["dve"] + cost_dve, acc["act"]
        ):
            acc["act"] += cost_act
            return "act"
        acc["dve"] += cost_dve
        return "dve"

    def emit_exp(s_t, w, toff, diag, force_act=False):
        """exp both halves of a combined score tile [t0: 0..w, t1:
        toff..toff+w]; merged into one instruction when the gap is small.
        Returns pT (bf16 view)."""
        ca, ma = _act_cost(w, toff)
        cv, mv = _dve_cost(w, toff)
        if force_act:
            acc["act"] += ca
            eng = "act"
        else:
            eng = pick_engine(ca, cv)
        merged = ma if eng == "act" else mv
        ranges = [(0, toff + w)] if merged else [(0, w), (toff, toff + w)]
        if eng == "act":
            pT = p_pool.tile([P, 2 * QP], BF16, tag="pa", name="pa")
            for lo, hi in ranges:
                nc.scalar.activation(pT[:, lo:hi], s_t[:, lo:hi], Exp, scale=SCALE)
            pTb = pT
        else:
            pTi = p_pool.tile([P, 2 * QP], I16, tag="pi", name="pi")
            for lo, hi in ranges:
                nc.vector.tensor_scalar(
                    out=pTi[:, lo:hi], in0=s_t[:, lo:hi],
                    scalar1=float(SCH_A), scalar2=float(SCH_B),
                    op0=mybir.AluOpType.mult, op1=mybir.AluOpType.add,
                )
            pTb = pTi.bitcast(BF16)
        if diag:
            # zero the strictly-upper triangle of the diagonal 128-block
            # (query < key) for both types
            for lo in (0, toff):
                nc.gpsimd.affine_select(
                    out=pTb[:, lo:lo + P], in_=pTb[:, lo:lo + P],
                    compare_op=mybir.AluOpType.is_ge, fill=0.0,
                    base=0, channel_multiplier=-1, pattern=[[1, P]],
                )
        return pTb

    def emit_copy(pv, g, pss, jlo, jhi, last):
        """pv accumulators for j-tiles [jlo, jhi) of this pass -> SBUF bf16
        -> DRAM (incl denominator col 64; the host divides)."""
        n = (jhi - jlo) * DP1
        ob = ob_pool.tile([P, 2, n], BF16, tag=f"ob{n}", name="ob")
        pv_v = pv[:, :, jlo * DP1:jhi * DP1]
        cost_a = (2 * n + 352) / 1.2
        cost_v = 2 * n * 1.04 + 150
        if pick_engine(cost_a, cost_v) == "act":
            nc.scalar.activation(ob, pv_v, Copy)
        else:
            nc.vector.tensor_copy(out=ob, in_=pv_v)
        eng = nc.scalar if last else (nc.sync if pss % 2 == 0 else nc.scalar)
        eng.dma_start(
            out=out[:, g, pss, :, jlo * DP1:jhi * DP1], in_=ob
        )

    # one flat, globally-pipelined schedule over all (pair, pass, key-tile)
    # chunks: PV matmuls trail two chunks behind their exp (so PE
    # instructions enter the queue with satisfied deps) and are interleaved
    # between the next chunk's score matmuls so each PV LDWEIGHTS can
    # prefetch under a long score stream.
    class Pass:
        def __init__(self, g, pss):
            self.g, self.pss = g, pss
            self.q0, self.j0 = pss * QP, 4 * pss
            self.kmax = 4 * (pss + 1)
            # one PV accumulator tile for both types: t0 in bank 0 (cols
            # 0:260), t1 in bank 1 (cols 512:772)
            self.pv = pv_pool.tile([P, 2, QP], F32, tag="pv", name="pv")
            self.started = [False, False]

        def emit_pv(self, t, k, qlo, w, toff, pTb):
            a = 2 * self.g + t
            js = list(range(max(self.j0, k), self.j0 + 4))
            if js and js[0] == k:
                # the j==k matmul reads the affine_select-masked diag block;
                # emit it LAST so the independent j>k matmuls are not stuck
                # behind its GPSIMD dependency in the PE FIFO
                js = js[1:] + [js[0]]
            for j in js:
                col = P * j - qlo + t * toff
                first = not self.started[t]
                self.started[t] = True
                nc.tensor.matmul(
                    self.pv[:, t, (j - self.j0) * DP1:(j - self.j0 + 1) * DP1],
                    lhsT=pTb[:, col:col + P],
                    rhs=ve_sb[:, a, k, :],
                    start=first,
                    stop=(k == j),
                    skip_group_check=True,
                )

    chunks = []
    for g in range(PAIRS):
        for pss in range(NPASS):
            for k in range(4 * (pss + 1)):
                chunks.append((g, pss, k))

    pend = []
    cur_pass = None

    def pop_pv(half):
        """Emit one type's trailing PV matmuls (half 0 -> t0, 1 -> t1 and
        retire); called between score segments of the current chunk."""
        if not pend:
            return
        pa, kk, qq, ww, to, pp, done = pend[0]
        pa.emit_pv(half, kk, qq, ww, to, pp)
        if half == 1:
            pend.pop(0)
            last = pa.g == PAIRS - 1 and pa.pss == NPASS - 1
            if last and kk == pa.kmax - 2:
                # final pass: j-tiles 0..2 are already complete after the
                # second-to-last key-tile; flush them early so only a tiny
                # copy+DMA trails the very last matmul
                emit_copy(pa.pv, pa.g, pa.pss, 0, 3, False)
            if done:
                # final key-tile of its pass: accumulators complete, flush
                if last:
                    emit_copy(pa.pv, pa.g, pa.pss, 3, 4, True)
                else:
                    emit_copy(pa.pv, pa.g, pa.pss, 0, 4, False)

    for g, pss, k in chunks:
        if cur_pass is None or (cur_pass.g, cur_pass.pss) != (g, pss):
            cur_pass = Pass(g, pss)
        q0 = pss * QP
        qlo = max(q0, P * k)
        w = q0 + QP - qlo
        diag = qlo == P * k
        s_t = s_pool.tile([P, 2 * QP], F32, tag="s", name="s")
        fp8 = USE_FP8 and pss >= 2
        # bf16 chunks pack gap-free: t0 at [0:w], t1 at [w:2w], with matmul
        # outputs split at absolute 512-col PSUM bank boundaries (first
        # matmul into each bank carries start=True, which clears the whole
        # bank's has_written; followers use False). fp8 DoubleRow matmuls
        # fault on such start=False continuation segments, so fp8 chunks use
        # 512-aligned placement (t1 at [512:512+w], a gap for partial widths).
        toff = QP if fp8 else w
        if fp8:
            segs = [(0, 0, w, True), (1, QP, QP + w, True)]
        else:
            segs = [(0, 0, w, True)]
            lo = w
            while lo < 2 * w:
                hi = min(2 * w, (lo // QP + 1) * QP)
                segs.append((1, lo, hi, lo % QP == 0))
                lo = hi
        emitted_pv = 0
        for t, lo, hi, bank_first in segs:
            c0 = qlo + lo - t * toff
            c1 = qlo + hi - t * toff
            if fp8:
                nc.tensor.matmul(
                    s_t[:, lo:hi],
                    lhsT=kt8_sb[32 * t:32 * (t + 1), g, :, P * k:P * (k + 1)],
                    rhs=qt8_sb[32 * t:32 * (t + 1), g, :, c0 - HALF:c1 - HALF],
                    start=bank_first,
                    stop=True,
                    perf_mode=mybir.MatmulPerfMode.DoubleRow,
                    skip_group_check=True,
                )
            else:
                nc.tensor.matmul(
                    s_t[:, lo:hi],
                    lhsT=kt_sb[:, t, g, P * k:P * (k + 1)],
                    rhs=qt_sb[:, g, c0:c1],
                    start=bank_first,
                    stop=True,
                    skip_group_check=True,
                )
            # interleave the trailing PV work between score segments
            if len(pend) == 5 and emitted_pv < 2:
                pop_pv(emitted_pv)
                emitted_pv += 1
        while len(pend) == 5 and emitted_pv < 2:
            pop_pv(emitted_pv)
            emitted_pv += 1
        # queries 0:511 average over few softmax terms, so the Schraudolph
        # ~3% element error would not cancel there: keep the first key-tile
        # of pass 0 on the exact ACT exp
        pTb = emit_exp(s_t, w, toff, diag, force_act=(pss == 0 and k == 0))
        pend.append(
            (cur_pass, k, qlo, w, toff, pTb, k == cur_pass.kmax - 1)
        )
    while pend:
        pop_pv(0)
        pop_pv(1)


def _host_shard(inputs):
    """Build the 8 per-core input maps from full inputs (host-side numpy)."""
    q_t = np.asarray(inputs["queries_time"], dtype=np.float32)
    k_t = np.asarray(inputs["keys_time"], dtype=np.float32)
    v_t = np.asarray(inputs["values_time"], dtype=np.float32)
    q_c = np.asarray(inputs["queries_channel"], dtype=np.float32)
    k_c = np.asarray(inputs["keys_channel"], dtype=np.float32)
    v_c = np.asarray(inputs["values_channel"], dtype=np.float32)

    bf16 = ml_dtypes.bfloat16
    fp8 = ml_dtypes.float8_e4m3
    in_maps = []
    for c in range(NCORES):
        vem = np.empty((P, NATT, NKT, DP1), np.float32)
        qtm = np.empty((P, PAIRS, L), np.float32)
        ktm = np.zeros((P, 2, PAIRS, L), np.float32)
        qt8m = np.empty((64, PAIRS, 2, HALF), np.float32)
        kt8m = np.empty((64, PAIRS, 2, L), np.float32)
        for g in range(PAIRS):
            p = PAIRS * c + g
            b, h = divmod(p, H)
            qtm[:64, g, :] = q_t[b, :, h, :].T
            qtm[64:, g, :] = q_c[b, :, h, :].T
            ktm[:64, 0, g, :] = k_t[b, :, h, :].T
            ktm[64:, 1, g, :] = k_c[b, :, h, :].T
            if USE_FP8:
                for t, (qf, kf) in enumerate(((q_t, k_t), (q_c, k_c))):
                    # E-index e -> partition 32*t + e%32, sub-row e//32
                    qT = qf[b, HALF:, h, :].T.reshape(2, 32, HALF)
                    kT = kf[b, :, h, :].T.reshape(2, 32, L)
                    qt8m[32 * t:32 * (t + 1), g] = qT.transpose(1, 0, 2)
                    kt8m[32 * t:32 * (t + 1), g] = kT.transpose(1, 0, 2)
            for t, v_full in enumerate((v_t, v_c)):
                a = 2 * g + t
                vem[:, a, :, :D] = (
                    v_full[b, :, h, :].reshape(NKT, P, D).transpose(1, 0, 2)
                )
                vem[:, a, :, D] = 1.0
        m = {
            "qt": np.ascontiguousarray(qtm).astype(bf16),
            "kt": np.ascontiguousarray(ktm).astype(bf16),
            "ve": np.ascontiguousarray(vem).astype(bf16),
        }
        if USE_FP8:
            m["qt8"] = np.ascontiguousarray(qt8m).astype(fp8)
            m["kt8"] = np.ascontiguousarray(kt8m).astype(fp8)
        in_maps.append(m)
    return in_maps


def _run(in_maps, trace=False):
    if "nc" not in _CACHE:
        _CACHE["nc"] = _build_nc()
    return run_bass_kernel_spmd(
        _CACHE["nc"], in_maps, core_ids=list(range(NCORES)), trace=trace
    )


def kernel(**inputs):
    in_maps = _host_shard(inputs)
    res = _run(in_maps, trace=False)
    v_time = np.empty((B, L, H, D), np.float32)
    v_chan = np.empty((B, L, H, D), np.float32)
    for c in range(NCORES):
        o = np.asarray(res.results[c]["out"]).astype(np.float32)
        # [P, PAIRS, NPASS, 2, 4*DP1] -> [P, PAIRS, 2, NKT, DP1]
        o = o.reshape(P, PAIRS, NPASS, 2, 4, DP1)
        o = o.transpose(1, 3, 0, 2, 4, 5).reshape(PAIRS, 2, P, NKT, DP1)
        o = o[..., :D] / o[..., D:DP1]  # host-side softmax normalization
        for g in range(PAIRS):
            p = PAIRS * c + g
            b, h = divmod(p, H)
            # q = 128*j + qq lives at o[g, t, qq, j, :]
            v_time[b, :, h, :] = o[g, 0].transpose(1, 0, 2).reshape(L, D)
            v_chan[b, :, h, :] = o[g, 1].transpose(1, 0, 2).reshape(L, D)
    return v_time, v_chan
